# revision 1
# baseline (speedup 1.0000x reference)
"""Trainium2 Bass kernel for CausalWanSelfAttention (block-causal attention with
RMSNorm'd+RoPE'd q/k), distributed over 8 NeuronCores via SPMD.

Sharding:
  - Token quarters (tiles 4/4/4/5 of 128): cores 0-3 compute the q projection
    (full 1536 dims) for their quarter, cores 4-7 the k projection. Every core
    also computes a [quarter x 768] block of v. RMSNorm and RoPE are local
    (token-major layout).
  - Exchange #1 is an AllToAll: each producer routes, per destination core,
    only that core's 2 head-slots (q/k rows d-major via PE transposes, v
    columns). Consumers then hold [head, all-tokens] q/k/v for their slots.
  - Attention: 16 head-slots (2 full causal triangles per core; the 4
    duplicate slots on cores 4-7 are discarded). Scores are computed
    transposed (sT[keys,q] = kT.T @ qT) so P^T feeds the V-matmul directly;
    the softmax denominator comes from a ones-vector matmul; no running max
    (scores are O(1) after RMS norm).
  - Exchange #2 is an AllToAll of (oT, l) sliced by destination token block;
    each core runs the output projection for a [544-token x 768-out-dim]
    block; the host assembles the blocks.
All matmuls run in float32r (full-rate fp32 on the PE at free-dim>=256).
"""
import math
import sys

sys.path.insert(0, "/opt/trn_rl_repo")

import numpy as np

import concourse.bass as bass
import concourse.tile as tile
from concourse import bacc, mybir
from concourse.bass import ds
from concourse.masks import make_identity

F32 = mybir.dt.float32
F32R = mybir.dt.float32r
I32 = mybir.dt.int32
AF = mybir.ActivationFunctionType
ALU = mybir.AluOpType

# problem constants (hardcoded per contract)
P = 128
DIM = 1536
NH = 12
HD = 128
S = 2080
SPAD = 2176
NTL = 17
F_, H_, W_ = 4, 20, 26
EPS = 1e-6
N_CORES = 8

QTILES = [4, 4, 4, 5]          # token tiles per quarter
QSTART = [0, 512, 1024, 1536]  # token start per quarter
QLEN = [512, 512, 512, 640]
TQ = 640                       # uniform (padded) tokens per core
NT = 5                         # uniform token tiles per core
VHALF = 768

# A2A #1 shard layout per destination: [2 slots x 128 qk rows, 640 tokens]
# followed by [640 tokens, 2 slots x 128 v cols]
QK1 = 2 * P * TQ               # 163840
V1 = TQ * 2 * P                # 163840
SHARD1 = QK1 + V1              # 327680 floats (1.31 MB)
V1_ROWS = SHARD1 // 256        # rows of 256 in the flat [.., 256] view
V1_ROW0 = QK1 // 256           # v-part starts at row 640 within a shard

TOK_BLK = 544                  # O-proj tokens per core
NHALF = 768                    # O-proj out-dims per core
OT_ROWS = 129                  # 128 oT rows + 1 l row
SHARD2 = 2 * OT_ROWS * TOK_BLK
GROUPS = [(0, 3), (4, 7), (8, 11), (12, 16)]
SCALE = 1.0 / math.sqrt(HD)

_CACHED_NC = None


def _slot_head(c, slot):
    return c if slot == 0 else 8 + (c % 4)


def _head_dests(h):
    """Destination (core, slot) pairs that attend head h."""
    if h < 8:
        return [(h, 0)]
    return [(h - 8, 1), (h - 4, 1)]


def _chunks(total, step=512):
    out = []
    a = 0
    while a < total:
        out.append((a, min(step, total - a)))
        a += step
    return out


def _bank_chunks(off, n, bank=512):
    """Split [off, off+n) at absolute multiples of `bank` (PSUM bank size)."""
    out = []
    a = off
    end = off + n
    while a < end:
        b = min(end, (a // bank + 1) * bank)
        out.append((a, b - a))
        a = b
    return out


def build_nc():
    nc = bacc.Bacc("TRN2", target_bir_lowering=False, debug=False,
                   num_devices=N_CORES)

    x_my = nc.dram_tensor("x_my", [TQ, DIM], F32, kind="ExternalInput").ap()
    w_proj = nc.dram_tensor("w_proj", [DIM, DIM], F32, kind="ExternalInput").ap()
    wv_half = nc.dram_tensor("wv_half", [DIM, VHALF], F32, kind="ExternalInput").ap()
    wo_slice = nc.dram_tensor("wo_slice", [DIM, NHALF], F32, kind="ExternalInput").ap()
    ang_in = nc.dram_tensor("ang_in", [TQ, 128], F32, kind="ExternalInput").ap()
    tabs = nc.dram_tensor("tabs", [1, 32], I32, kind="ExternalInput").ap()
    outT = nc.dram_tensor("outT", [NHALF, TOK_BLK], F32, kind="ExternalOutput").ap()

    with tile.TileContext(nc) as tc:
        _body(tc, x_my, w_proj, wv_half, wo_slice, ang_in, tabs, outT)
    nc.compile()
    return nc


def _body(tc, *args):
    from contextlib import ExitStack
    with ExitStack() as es:
        const = es.enter_context(tc.tile_pool(name="const", bufs=1))
        dram = es.enter_context(tc.tile_pool(name="dram", bufs=1, space="DRAM"))
        shared = es.enter_context(tc.tile_pool(name="shared", bufs=1, space="DRAM"))
        _body2(tc, const, dram, shared, *args)


def _body2(tc, const, dram, shared,
           x_my, w_proj, wv_half, wo_slice, ang_in, tabs, outT):
    nc = tc.nc

    ident = const.tile([P, P], F32)
    make_identity(nc, ident)
    ones_f32 = const.tile([P, 1], F32)
    nc.vector.memset(ones_f32, 1.0)
    ones_col = const.tile([P, 1], F32R)
    nc.vector.tensor_copy(ones_col, ones_f32)
    eps_t = const.tile([P, 1], F32)
    nc.vector.memset(eps_t, EPS)
    tab_sb = const.tile([1, 32], I32)
    nc.sync.dma_start(out=tab_sb, in_=tabs)

    # rope tables: ang_in = [sin-angles | cos-angles], both reduced to [-pi,pi)
    cos_sb = const.tile([P, NT, 64], F32)
    sin_sb = const.tile([P, NT, 64], F32)
    ang_sb = const.tile([P, NT, 128], F32)
    nc.sync.dma_start(out=ang_sb, in_=ang_in.rearrange("(a p) c -> p a c", p=P))
    nc.scalar.activation(sin_sb[:, :, :], ang_sb[:, :, 0:64], AF.Sin)
    nc.scalar.activation(cos_sb[:, :, :], ang_sb[:, :, 64:128], AF.Sin)

    # collective buffers
    send1 = dram.tile([N_CORES, SHARD1], F32)
    recv1 = dram.tile([N_CORES, SHARD1], F32)
    send2 = dram.tile([N_CORES, 2, OT_ROWS, TOK_BLK], F32)
    recv2 = dram.tile([N_CORES, 2 * OT_ROWS * TOK_BLK], F32)
    rl_dram = dram.tile([NH, TOK_BLK], F32)

    # views of send1
    s1_qk = [send1[d:d + 1, 0:QK1].rearrange("one (r t) -> (one r) t", t=TQ)
             for d in range(N_CORES)]
    s1_v = [send1[d:d + 1, QK1:SHARD1].rearrange("one (t v) -> (one t) v", v=256)
            for d in range(N_CORES)]

    # ---------------- Phase A+B+C: xT, q/k projection + RMS + rope, v ----------------
    with tc.tile_pool(name="resident", bufs=1) as res, \
         tc.tile_pool(name="xtiles", bufs=2) as xtiles, \
         tc.tile_pool(name="wpool", bufs=2) as wpool, \
         tc.tile_pool(name="work", bufs=4) as work, \
         tc.tile_pool(name="evict", bufs=4) as evict, \
         tc.tile_pool(name="psA", bufs=3, space="PSUM") as psA, \
         tc.tile_pool(name="psT", bufs=2, space="PSUM") as psT:

        xT = res.tile([P, 12, TQ], F32R)          # x^T, d-major (3.9MB)
        q_raw = res.tile([P, NT, DIM], F32)       # projection out, token-major

        # A: load x tiles, PE-transpose into xT
        for t in range(NT):
            x_t = xtiles.tile([P, DIM], F32, tag="x_t")
            nc.sync.dma_start(out=x_t, in_=x_my[t * P:(t + 1) * P, :])
            for k in range(12):
                tp = psT.tile([P, P], F32, tag="tp")
                nc.tensor.transpose(tp, x_t[:, k * P:(k + 1) * P], ident)
                nc.vector.tensor_copy(xT[:, k, t * P:(t + 1) * P], tp)

        # B: q (or k) projection, n-chunk outer so weights stream once
        ssq = work.tile([P, NT, 3], F32, tag="ssq", bufs=1)
        for n in range(3):
            w_n = wpool.tile([P, 12, 512], F32R, tag="w_n")
            nc.sync.dma_start(
                out=w_n,
                in_=w_proj[:, n * 512:(n + 1) * 512]
                .rearrange("(k p) d -> p k d", p=P).bitcast(F32R))
            for t in range(NT):
                mm_ps = psA.tile([P, 512], F32, tag="mm")
                for k in range(12):
                    nc.tensor.matmul(mm_ps, xT[:, k, t * P:(t + 1) * P],
                                     w_n[:, k, :], start=(k == 0), stop=(k == 11))
                sq_scr = work.tile([P, 512], F32, tag="sq_scr")
                nc.scalar.activation(sq_scr, mm_ps, AF.Square,
                                     accum_out=ssq[:, t, n:n + 1])
                nc.vector.tensor_copy(q_raw[:, t, n * 512:(n + 1) * 512], mm_ps)

        # RMS + rope per token tile (in-place on q_raw)
        for t in range(NT):
            s01 = work.tile([P, 1], F32, tag="s01")
            nc.vector.tensor_tensor(s01, ssq[:, t, 0:1], ssq[:, t, 1:2], ALU.add)
            stot = work.tile([P, 1], F32, tag="stot")
            nc.vector.tensor_tensor(stot, s01, ssq[:, t, 2:3], ALU.add)
            sq_t = work.tile([P, 1], F32, tag="sq_t")
            nc.scalar.activation(sq_t, stot, AF.Sqrt, bias=eps_t,
                                 scale=1.0 / DIM)
            rsq = work.tile([P, 1], F32, tag="rsq")
            nc.vector.reciprocal(rsq, sq_t)
            crsq = work.tile([P, 64], F32, tag="crsq")
            srsq = work.tile([P, 64], F32, tag="srsq")
            nc.vector.tensor_scalar_mul(crsq, cos_sb[:, t, :], rsq)
            nc.vector.tensor_scalar_mul(srsq, sin_sb[:, t, :], rsq)
            cb = bass.AP(tensor=crsq.tensor, offset=crsq.offset,
                         ap=[crsq.ap[0], [0, NH], crsq.ap[1]])
            sbb = bass.AP(tensor=srsq.tensor, offset=srsq.offset,
                          ap=[srsq.ap[0], [0, NH], srsq.ap[1]])
            qh = q_raw[:, t, :].rearrange("p (h c two) -> p h c two", h=NH, two=2)
            qe = qh[:, :, :, 0]
            qo = qh[:, :, :, 1]
            tA = work.tile([P, NH, 64], F32, tag="tA")
            tB = work.tile([P, NH, 64], F32, tag="tB")
            tC = work.tile([P, NH, 64], F32, tag="tC")
            tD = work.tile([P, NH, 64], F32, tag="tD")
            nc.vector.tensor_tensor(tA, qe, cb, ALU.mult)
            nc.vector.tensor_tensor(tB, qo, sbb, ALU.mult)
            nc.vector.tensor_tensor(tC, qe, sbb, ALU.mult)
            nc.vector.tensor_tensor(tD, qo, cb, ALU.mult)
            nc.vector.tensor_tensor(qe, tA, tB, ALU.subtract)
            nc.vector.tensor_tensor(qo, tC, tD, ALU.add)

        # transpose roped q/k into a per-head stage, then one DMA per route
        for h in range(12):
            stage_h = evict.tile([P, TQ], F32, tag="stage_h", bufs=3)
            for t in range(NT):
                tp = psT.tile([P, P], F32, tag="tp")
                nc.tensor.transpose(tp, q_raw[:, t, h * P:(h + 1) * P], ident)
                nc.vector.tensor_copy(stage_h[:, t * P:(t + 1) * P], tp)
            for (d, sl) in _head_dests(h):
                nc.sync.dma_start(
                    out=s1_qk[d][sl * P:(sl + 1) * P, :], in_=stage_h)

        # C: v projection [TQ, VHALF] into v_sb, then route columns per dest
        v_sb = res.tile([P, NT, VHALF], F32)
        for n0, nn in ((0, 512), (512, 256)):
            wv_n = wpool.tile([P, 12, 512], F32R, tag="w_n")
            nc.sync.dma_start(
                out=wv_n[:, :, 0:nn],
                in_=wv_half[:, n0:n0 + nn]
                .rearrange("(k p) d -> p k d", p=P).bitcast(F32R))
            for t in range(NT):
                mm_ps = psA.tile([P, 512], F32, tag="mm")
                for k in range(12):
                    nc.tensor.matmul(mm_ps[:, 0:nn], xT[:, k, t * P:(t + 1) * P],
                                     wv_n[:, k, 0:nn], start=(k == 0), stop=(k == 11))
                nc.vector.tensor_copy(v_sb[:, t, n0:n0 + nn], mm_ps[:, 0:nn])

        # v routing: local head column lh is head lh on a q-core and head
        # 6+lh on a k-core. Write BOTH halves' destination patterns
        # statically - a consumer only reads the shards of the 4 ranks of the
        # correct half for each slot, so wrong-half writes are dead data.
        for lh in range(6):
            dests = set(_head_dests(lh)) | set(_head_dests(6 + lh))
            for (d, sl) in sorted(dests):
                nc.sync.dma_start(
                    out=s1_v[d].rearrange("(a p) v -> p a v", p=P)
                    [:, :, sl * P:(sl + 1) * P],
                    in_=v_sb[:, :, lh * P:(lh + 1) * P])

    # ---------------- A2A #1 ----------------
    nc.gpsimd.collective_compute(
        "AllToAll", ALU.bypass, replica_groups=[list(range(N_CORES))],
        ins=[send1.opt()], outs=[recv1.opt()])

    r1_qk = [recv1[r:r + 1, 0:QK1].rearrange("one (w t) -> (one w) t", t=TQ)
             for r in range(N_CORES)]
    r1_v2d = recv1.rearrange("r c -> (r c)").rearrange("(a v) -> a v", v=256)

    # ---------------- Phase D: attention, 2 head slots ----------------
    with tc.tile_pool(name="attn", bufs=1) as attn, \
         tc.tile_pool(name="ptp", bufs=3) as ptp, \
         tc.tile_pool(name="aev", bufs=2) as aev, \
         tc.tile_pool(name="psO", bufs=1, space="PSUM") as psO, \
         tc.tile_pool(name="psS", bufs=2, space="PSUM") as psS, \
         tc.tile_pool(name="psL", bufs=1, space="PSUM") as psL:
        for slot in range(2):
            qTc = attn.tile([P, NTL * P], F32R, tag="qTc", bufs=2)
            kTc = attn.tile([P, NTL * P], F32R, tag="kTc", bufs=2)
            Vc = attn.tile([P, NTL, P], F32R, tag="Vc", bufs=2)

            for r in range(4):
                tb = 4 * r * P
                nl = QTILES[r] * P
                nc.sync.dma_start(
                    out=qTc[:, tb:tb + nl],
                    in_=r1_qk[r][slot * P:(slot + 1) * P, 0:nl].bitcast(F32R))
                nc.sync.dma_start(
                    out=kTc[:, tb:tb + nl],
                    in_=r1_qk[r + 4][slot * P:(slot + 1) * P, 0:nl].bitcast(F32R))
            with nc.gpsimd.register(f"vr_{slot}") as rr:
                for r in range(4):
                    idx = slot * 4 + r
                    nc.gpsimd.reg_load(rr, tab_sb[0:1, idx:idx + 1])
                    vrow = nc.gpsimd.snap(rr)
                    nc.gpsimd.dma_start(
                        out=Vc[:, 4 * r:4 * r + QTILES[r], :],
                        in_=r1_v2d[ds(vrow, QTILES[r] * P),
                                   slot * P:(slot + 1) * P]
                        .rearrange("(a p) d -> p a d", p=P).bitcast(F32R))

            for (t0, t1) in GROUPS:
                ng = (t1 - t0 + 1) * P
                oT_ps = psO.tile([P, ng], F32, tag="oT")
                l_ps = psL.tile([1, ng], F32, tag="l")
                for kt in range(t1 + 1):
                    c0 = max(t0, kt)
                    off = (c0 - t0) * P
                    n = (t1 - c0 + 1) * P
                    sT_ps = psS.tile([P, n], F32, tag="sT")
                    for (ja, jn) in _chunks(n):
                        nc.tensor.matmul(sT_ps[:, ja:ja + jn],
                                         kTc[:, kt * P:(kt + 1) * P],
                                         qTc[:, c0 * P + ja:c0 * P + ja + jn],
                                         start=True, stop=True)
                    PT = ptp.tile([P, n], F32R, tag="PT")
                    nc.scalar.activation(PT, sT_ps, AF.Exp, scale=SCALE)
                    if kt == 16:
                        # zero pad-key rows 32..128 (memset can't write f32r;
                        # a base partition of 32 may span at most 32 rows)
                        nc.vector.tensor_scalar_mul(PT[32:64, :], PT[32:64, :], 0.0)
                        nc.vector.tensor_scalar_mul(PT[64:P, :], PT[64:P, :], 0.0)
                    # accumulation groups are per PSUM bank: a bank's last
                    # write happens at kt == its highest column tile
                    for (ja, jn) in _bank_chunks(off, n):
                        bank = ja // 512
                        fin = (kt == min(t1, t0 + 4 * bank + 3))
                        nc.tensor.matmul(oT_ps[:, ja:ja + jn],
                                         Vc[:, kt, :], PT[:, ja - off:ja - off + jn],
                                         start=(kt == 0), stop=fin)
                        nc.tensor.matmul(l_ps[:, ja:ja + jn],
                                         ones_col, PT[:, ja - off:ja - off + jn],
                                         start=(kt == 0), stop=fin)
                # evict group results, slicing into destination token blocks
                oT_sb = aev.tile([P, ng], F32, tag="oT_sb")
                nc.vector.tensor_copy(oT_sb, oT_ps)
                l_sb = aev.tile([1, ng], F32, tag="l_sb")
                nc.vector.tensor_copy(l_sb, l_ps)
                g0 = t0 * P
                for j in range(4):
                    a = max(g0, j * TOK_BLK)
                    b = min(g0 + ng, (j + 1) * TOK_BLK)
                    if a >= b:
                        continue
                    for dd in (j, j + 4):
                        nc.sync.dma_start(
                            out=send2[dd, slot, 0:P, a - j * TOK_BLK:b - j * TOK_BLK],
                            in_=oT_sb[:, a - g0:b - g0])
                        nc.sync.dma_start(
                            out=send2[dd, slot, P:P + 1,
                                      a - j * TOK_BLK:b - j * TOK_BLK],
                            in_=l_sb[:, a - g0:b - g0])

    # ---------------- A2A #2 ----------------
    nc.gpsimd.collective_compute(
        "AllToAll", ALU.bypass, replica_groups=[list(range(N_CORES))],
        ins=[send2.opt()], outs=[recv2.opt()])

    r2 = recv2.rearrange("r (s o t) -> r s o t", s=2, o=OT_ROWS)

    def head_src(h):
        return (h, 0) if h < 8 else (h - 8, 1)

    # ---------------- Phase E: output projection ----------------
    with tc.tile_pool(name="oproj", bufs=1) as op, \
         tc.tile_pool(name="owork", bufs=3) as ow, \
         tc.tile_pool(name="psP", bufs=2, space="PSUM") as psP:

        wo_sb = op.tile([P, 12, NHALF], F32R)
        nc.sync.dma_start(out=wo_sb,
                          in_=wo_slice.rearrange("(k p) d -> p k d", p=P)
                          .bitcast(F32R))

        l_all = op.tile([NH, TOK_BLK], F32)
        oT_asm = op.tile([P, NH, TOK_BLK], F32R)

        for h in range(NH):
            rk, sl = head_src(h)
            nc.sync.dma_start(out=l_all[h:h + 1, :],
                              in_=r2[rk, sl, P:P + 1, :])
        rl = op.tile([NH, TOK_BLK], F32)
        nc.vector.reciprocal(rl, l_all)
        nc.sync.dma_start(out=rl_dram, in_=rl)

        for h in range(NH):
            rk, sl = head_src(h)
            oTh = ow.tile([P, TOK_BLK], F32, tag="oTh")
            nc.sync.dma_start(out=oTh, in_=r2[rk, sl, 0:P, :])
            rlb = ow.tile([P, TOK_BLK], F32, tag="rlb")
            rl_bc = bass.AP(tensor=rl_dram.tensor,
                            offset=rl_dram.offset + h * TOK_BLK,
                            ap=[[0, P], [1, TOK_BLK]])
            nc.sync.dma_start(out=rlb, in_=rl_bc)
            nc.vector.tensor_tensor(oT_asm[:, h, :], oTh, rlb, ALU.mult)

        for m in range(6):
            ps = psP.tile([P, TOK_BLK], F32, tag="psP")
            for (ja, jn) in _chunks(TOK_BLK):
                for k in range(NH):
                    nc.tensor.matmul(ps[:, ja:ja + jn],
                                     wo_sb[:, k, m * P:(m + 1) * P],
                                     oT_asm[:, k, ja:ja + jn],
                                     start=(k == 0), stop=(k == NH - 1))
            oev = ow.tile([P, TOK_BLK], F32, tag="oev")
            nc.vector.tensor_copy(oev, ps)
            nc.sync.dma_start(out=outT[m * P:(m + 1) * P, :], in_=oev)


# ======================= host side =======================

def _expected_mask():
    blk = np.arange(SPAD) // P
    return (blk[:, None] >= blk[None, :]) & (np.arange(SPAD)[None, :] < S)


def _host_prep(x, freqs, wq, wk, wv, wo):
    """Build the 8 per-core input maps."""
    x_pad = np.zeros((SPAD, DIM), np.float32)
    x_pad[:S] = x[0]

    # rope angle table (pure gather from freqs)
    t = np.arange(S)
    fi = t // (H_ * W_)
    hi = (t % (H_ * W_)) // W_
    wi = t % W_
    ang = np.zeros((SPAD, 64), np.float32)
    ang[:S, 0:22] = freqs[fi, 0:22]
    ang[:S, 22:43] = freqs[hi, 22:43]
    ang[:S, 43:64] = freqs[wi, 43:64]

    in_maps = []
    for c in range(N_CORES):
        qr = c % 4
        x_my = np.zeros((TQ, DIM), np.float32)
        x_my[:QLEN[qr]] = x_pad[QSTART[qr]:QSTART[qr] + QLEN[qr]]
        ang_q = ang[QSTART[qr]:QSTART[qr] + QLEN[qr]]
        ang_my = np.zeros((TQ, 128), np.float32)
        twopi = 2.0 * math.pi
        # reduce into [-pi, pi): the ACT Sin LUT only covers |x| < 4
        ang_my[:QLEN[qr], 0:64] = np.mod(ang_q + math.pi, twopi) - math.pi
        ang_my[:QLEN[qr], 64:128] = np.mod(ang_q + math.pi / 2.0 + math.pi,
                                           twopi) - math.pi

        tabs = np.zeros((1, 32), np.int32)
        # consumer: v source rank row offsets per slot, per quarter
        for slot in range(2):
            head = _slot_head(c, slot)
            vbase = 0 if head < 6 else 4
            for r in range(4):
                tabs[0, slot * 4 + r] = (vbase + r) * V1_ROWS + V1_ROW0
        # producer: v column routing, 6 local heads x up to 2 dests
        half = 0 if c < 4 else 1
        for lh in range(6):
            h = half * 6 + lh
            dests = _head_dests(h)
            for j in range(2):
                base = 8 + 2 * (lh * 2 + j)
                if j < len(dests):
                    d, sl = dests[j]
                    tabs[0, base] = d * V1_ROWS + V1_ROW0
                    tabs[0, base + 1] = sl * P
                else:
                    tabs[0, base] = N_CORES * V1_ROWS + V1_ROW0  # dummy shard
                    tabs[0, base + 1] = 0

        in_maps.append({
            "x_my": x_my,
            "w_proj": np.ascontiguousarray(wq if c < 4 else wk),
            "wv_half": np.ascontiguousarray(
                wv[:, :VHALF] if c < 4 else wv[:, VHALF:]),
            "wo_slice": np.ascontiguousarray(
                wo[:, (c // 4) * NHALF:(c // 4 + 1) * NHALF]),
            "ang_in": ang_my,
            "tabs": tabs,
        })
    return in_maps


def get_nc():
    global _CACHED_NC
    if _CACHED_NC is None:
        _CACHED_NC = build_nc()
    return _CACHED_NC


def kernel(x, freqs, wq, bq, wk, bk, wv, bv, wo, bo, gq, gk,
           seq_lens, grid_sizes, mask, _run=None):
    x = np.asarray(x, np.float32)
    freqs = np.asarray(freqs, np.float32)
    wq, wk, wv, wo = (np.asarray(w, np.float32) for w in (wq, wk, wv, wo))

    assert x.shape == (1, S, DIM)
    assert int(np.asarray(seq_lens)[0]) == S, "teacher-forcing path not implemented"
    assert tuple(np.asarray(grid_sizes)[0]) == (F_, H_, W_)
    for b in (bq, bk, bv, bo):
        assert not np.any(np.asarray(b)), "nonzero bias not supported"
    for g in (gq, gk):
        assert np.all(np.asarray(g) == 1.0), "non-unit norm gain not supported"
    assert np.array_equal(np.asarray(mask), _expected_mask()), "unexpected mask"

    in_maps = _host_prep(x, freqs, wq, wk, wv, wo)

    if _run is None:
        from concourse.bass_utils import run_bass_kernel_spmd
        nc = get_nc()
        res = run_bass_kernel_spmd(nc, in_maps, list(range(N_CORES)))
        outs = [res.results[c]["outT"] for c in range(N_CORES)]
    else:
        outs = _run(in_maps)

    out = np.zeros((SPAD, DIM), np.float32)
    for c in range(N_CORES):
        r0 = (c % 4) * TOK_BLK
        n0 = (c // 4) * NHALF
        out[r0:r0 + TOK_BLK, n0:n0 + NHALF] = outs[c].T
    return out[:S][None]



# revision 6
# speedup vs baseline: 1.3365x; 1.3365x over previous
"""Trainium2 Bass kernel for CausalWanSelfAttention (block-causal attention with
RMSNorm'd+RoPE'd q/k), distributed over 8 NeuronCores via SPMD.

v2 vs baseline: all exchanges and matmul operands in bf16 (host casts x and
weights; PSUM accumulation stays fp32), the two big AllToAlls are split into
four smaller ones that overlap with compute (v-projection covers the q/k
exchange, the score pass covers the v exchange, slot-1 attention covers the
slot-0 output exchange, and the first 8 heads of the output projection cover
the slot-1 exchange), attention is restructured into a score pass (all
exp'd P^T tiles stored in SBUF) followed by a PV pass, and the softmax
normalization happens on the attention core (1/l broadcast across partitions
via a ones-matmul) so only 128 oT rows are exchanged.

Sharding (unchanged):
  - Token quarters (tiles 4/4/4/5 of 128): cores 0-3 compute the q projection
    (full 1536 dims) for their quarter, cores 4-7 the k projection. Every core
    also computes a [quarter x 768] block of v. RMSNorm and RoPE are local
    (token-major layout, fp32, cast to bf16 at the transpose-evict).
  - Attention: 16 head-slots (2 full causal triangles per core; the 4
    duplicate slots on cores 4-7 are discarded). Scores are computed
    transposed (sT[keys,q] = kT.T @ qT) so P^T feeds the V-matmul directly;
    the softmax denominator comes from a ones-vector matmul.
  - Each core runs the output projection for a [544-token x 768-out-dim]
    block; the host assembles the blocks.
"""
import math
import sys

sys.path.insert(0, "/opt/trn_rl_repo")

import numpy as np

import concourse.bass as bass
import concourse.tile as tile
from concourse import bacc, mybir
from concourse.bass import ds
from concourse.masks import make_identity

F32 = mybir.dt.float32
BF = mybir.dt.bfloat16
I32 = mybir.dt.int32
AF = mybir.ActivationFunctionType
ALU = mybir.AluOpType

# problem constants (hardcoded per contract)
P = 128
DIM = 1536
NH = 12
HD = 128
S = 2080
SPAD = 2176
NTL = 17
F_, H_, W_ = 4, 20, 26
EPS = 1e-6
N_CORES = 8

QTILES = [4, 4, 4, 5]          # token tiles per quarter
QSTART = [0, 512, 1024, 1536]  # token start per quarter
QLEN = [512, 512, 512, 640]
TQ = 640                       # uniform (padded) tokens per core
NT = 5                         # uniform token tiles per core
VHALF = 768

TOK_BLK = 544                  # O-proj tokens per core
NHALF = 768                    # O-proj out-dims per core
GROUPS = [(0, 3), (4, 7), (8, 11), (12, 16)]
SCALE = 1.0 / math.sqrt(HD)

_CACHED_NC = None


def _slot_head(c, slot):
    return c if slot == 0 else 8 + (c % 4)


def _head_dests(h):
    """Destination (core, slot) pairs that attend head h."""
    if h < 8:
        return [(h, 0)]
    return [(h - 8, 1), (h - 4, 1)]


def _chunks(total, step=512):
    out = []
    a = 0
    while a < total:
        out.append((a, min(step, total - a)))
        a += step
    return out


def _bank_chunks(off, n, bank=512):
    """Split [off, off+n) at absolute multiples of `bank` (PSUM bank size)."""
    out = []
    a = off
    end = off + n
    while a < end:
        b = min(end, (a // bank + 1) * bank)
        out.append((a, b - a))
        a = b
    return out


def _pt_offsets():
    """Column offset of each (group, kt) P^T tile in the per-slot PT store."""
    offs = {}
    col = 0
    for (t0, t1) in GROUPS:
        for kt in range(t1 + 1):
            c0 = max(t0, kt)
            n = (t1 - c0 + 1) * P
            offs[(t0, kt)] = (col, c0, n)
            col += n
    return offs, col


PT_OFFS, PT_COLS = _pt_offsets()   # PT_COLS = 19584


def build_nc():
    nc = bacc.Bacc("TRN2", target_bir_lowering=False, debug=False,
                   num_devices=N_CORES)

    x_my = nc.dram_tensor("x_my", [TQ, DIM], BF, kind="ExternalInput").ap()
    w_proj = nc.dram_tensor("w_proj", [DIM, DIM], BF, kind="ExternalInput").ap()
    wv_half = nc.dram_tensor("wv_half", [DIM, VHALF], BF, kind="ExternalInput").ap()
    wo_slice = nc.dram_tensor("wo_slice", [DIM, NHALF], BF, kind="ExternalInput").ap()
    cs_in = nc.dram_tensor("cs_in", [TQ, 128], F32, kind="ExternalInput").ap()
    tabs = nc.dram_tensor("tabs", [1, 32], I32, kind="ExternalInput").ap()
    outT = nc.dram_tensor("outT", [NHALF, TOK_BLK], F32, kind="ExternalOutput").ap()

    with tile.TileContext(nc) as tc:
        _body(tc, x_my, w_proj, wv_half, wo_slice, cs_in, tabs, outT)
    nc.compile()
    return nc


def _body(tc, *args):
    from contextlib import ExitStack
    with ExitStack() as es:
        const = es.enter_context(tc.tile_pool(name="const", bufs=1))
        dram = es.enter_context(tc.tile_pool(name="dram", bufs=1, space="DRAM"))
        _body2(tc, const, dram, *args)


def _body2(tc, const, dram,
           x_my, w_proj, wv_half, wo_slice, cs_in, tabs, outT):
    nc = tc.nc

    ident = const.tile([P, P], BF)
    make_identity(nc, ident)
    ident32 = const.tile([P, P], F32)
    make_identity(nc, ident32)
    ones_bf = const.tile([P, 1], BF)
    nc.vector.memset(ones_bf, 1.0)
    ones_row = const.tile([1, P], BF)
    nc.vector.memset(ones_row, 1.0)
    eps_t = const.tile([P, 1], F32)
    nc.vector.memset(eps_t, EPS)
    tab_sb = const.tile([1, 32], I32)
    nc.sync.dma_start(out=tab_sb, in_=tabs)
    wo_sb = const.tile([P, 12, NHALF], BF)
    nc.sync.dma_start(out=wo_sb,
                      in_=wo_slice.rearrange("(k p) d -> p k d", p=P))

    # rope tables: cs_in = [cos | sin] per token, fp32
    cs_sb = const.tile([P, NT, 128], F32)
    nc.sync.dma_start(out=cs_sb, in_=cs_in.rearrange("(a p) c -> p a c", p=P))
    cos_sb = cs_sb[:, :, 0:64]
    sin_sb = cs_sb[:, :, 64:128]

    # collective buffers (bf16)
    send_qk = dram.tile([N_CORES, 2 * P, TQ], BF)
    recv_qk = dram.tile([N_CORES, 2 * P, TQ], BF)
    send_v = dram.tile([N_CORES, TQ, 256], BF)
    recv_v = dram.tile([N_CORES, TQ, 256], BF)
    send_o0 = dram.tile([N_CORES, P, TOK_BLK], BF)
    recv_o0 = dram.tile([N_CORES, P, TOK_BLK], BF)
    send_o1 = dram.tile([N_CORES, P, TOK_BLK], BF)
    recv_o1 = dram.tile([N_CORES, P, TOK_BLK], BF)

    # ---------------- Phase A+B: xT, q/k projection + RMS + rope ----------------
    with tc.tile_pool(name="resident", bufs=1) as res, \
         tc.tile_pool(name="xtiles", bufs=2) as xtiles, \
         tc.tile_pool(name="wpool", bufs=2) as wpool, \
         tc.tile_pool(name="work", bufs=4) as work, \
         tc.tile_pool(name="evict", bufs=4) as evict, \
         tc.tile_pool(name="psA", bufs=3, space="PSUM") as psA, \
         tc.tile_pool(name="psT", bufs=2, space="PSUM") as psT:

        xT = res.tile([P, 12, TQ], BF)            # x^T, d-major (2.0MB)
        q_raw = res.tile([P, NT, DIM], F32)       # projection out, token-major

        # A: load x tiles, PE-transpose into xT
        for t in range(NT):
            x_t = xtiles.tile([P, DIM], BF, tag="x_t")
            nc.sync.dma_start(out=x_t, in_=x_my[t * P:(t + 1) * P, :])
            for k in range(12):
                tp = psT.tile([P, P], BF, tag="tpx")
                nc.tensor.transpose(tp, x_t[:, k * P:(k + 1) * P], ident)
                nc.vector.tensor_copy(xT[:, k, t * P:(t + 1) * P], tp)

        # B: q (or k) projection, n-chunk outer so weights stream once
        ssq = work.tile([P, NT, 3], F32, tag="ssq", bufs=1)
        for n in range(3):
            w_n = wpool.tile([P, 12, 512], BF, tag="w_n")
            nc.sync.dma_start(
                out=w_n,
                in_=w_proj[:, n * 512:(n + 1) * 512]
                .rearrange("(k p) d -> p k d", p=P))
            for t in range(NT):
                mm_ps = psA.tile([P, 512], F32, tag="mm")
                for k in range(12):
                    nc.tensor.matmul(mm_ps, xT[:, k, t * P:(t + 1) * P],
                                     w_n[:, k, :], start=(k == 0), stop=(k == 11))
                sq_scr = work.tile([P, 512], F32, tag="sq_scr")
                nc.scalar.activation(sq_scr, mm_ps, AF.Square,
                                     accum_out=ssq[:, t, n:n + 1])
                nc.vector.tensor_copy(q_raw[:, t, n * 512:(n + 1) * 512], mm_ps)

        # RMS + rope per token tile (in-place on q_raw, fp32)
        for t in range(NT):
            s01 = work.tile([P, 1], F32, tag="s01")
            nc.vector.tensor_tensor(s01, ssq[:, t, 0:1], ssq[:, t, 1:2], ALU.add)
            stot = work.tile([P, 1], F32, tag="stot")
            nc.vector.tensor_tensor(stot, s01, ssq[:, t, 2:3], ALU.add)
            sq_t = work.tile([P, 1], F32, tag="sq_t")
            nc.scalar.activation(sq_t, stot, AF.Sqrt, bias=eps_t,
                                 scale=1.0 / DIM)
            rsq = work.tile([P, 1], F32, tag="rsq")
            nc.vector.reciprocal(rsq, sq_t)
            crsq = work.tile([P, 64], F32, tag="crsq")
            srsq = work.tile([P, 64], F32, tag="srsq")
            nc.vector.tensor_scalar_mul(crsq, cos_sb[:, t, :], rsq)
            nc.vector.tensor_scalar_mul(srsq, sin_sb[:, t, :], rsq)
            cb = bass.AP(tensor=crsq.tensor, offset=crsq.offset,
                         ap=[crsq.ap[0], [0, NH], crsq.ap[1]])
            sbb = bass.AP(tensor=srsq.tensor, offset=srsq.offset,
                          ap=[srsq.ap[0], [0, NH], srsq.ap[1]])
            qh = q_raw[:, t, :].rearrange("p (h c two) -> p h c two", h=NH, two=2)
            qe = qh[:, :, :, 0]
            qo = qh[:, :, :, 1]
            tA = work.tile([P, NH, 64], F32, tag="tA")
            tB = work.tile([P, NH, 64], F32, tag="tB")
            tC = work.tile([P, NH, 64], F32, tag="tC")
            tD = work.tile([P, NH, 64], F32, tag="tD")
            nc.vector.tensor_tensor(tA, qe, cb, ALU.mult)
            nc.vector.tensor_tensor(tB, qo, sbb, ALU.mult)
            nc.vector.tensor_tensor(tC, qe, sbb, ALU.mult)
            nc.vector.tensor_tensor(tD, qo, cb, ALU.mult)
            nc.vector.tensor_tensor(qe, tA, tB, ALU.subtract)
            nc.vector.tensor_tensor(qo, tC, tD, ALU.add)

        # transpose roped q/k into a per-head bf16 stage, then DMA per route
        for h in range(12):
            stage_h = evict.tile([P, TQ], BF, tag="stage_h", bufs=3)
            for t in range(NT):
                tp = psT.tile([P, P], F32, tag="tpq")
                nc.tensor.transpose(tp, q_raw[:, t, h * P:(h + 1) * P], ident32)
                nc.vector.tensor_copy(stage_h[:, t * P:(t + 1) * P], tp)
            for (d, sl) in _head_dests(h):
                nc.sync.dma_start(
                    out=send_qk[d, sl * P:(sl + 1) * P, :], in_=stage_h)

        # ---------------- A2A #1a: q/k (overlaps with v projection) -------
        nc.gpsimd.collective_compute(
            "AllToAll", ALU.bypass, replica_groups=[list(range(N_CORES))],
            ins=[send_qk.opt()], outs=[recv_qk.opt()])

        # C: v projection [TQ, VHALF] into v_sb, then route columns per dest
        v_sb = res.tile([P, NT, VHALF], BF)
        for n0, nn in ((0, 512), (512, 256)):
            wv_n = wpool.tile([P, 12, 512], BF, tag="w_n")
            nc.sync.dma_start(
                out=wv_n[:, :, 0:nn],
                in_=wv_half[:, n0:n0 + nn]
                .rearrange("(k p) d -> p k d", p=P))
            for t in range(NT):
                mm_ps = psA.tile([P, 512], F32, tag="mm")
                for k in range(12):
                    nc.tensor.matmul(mm_ps[:, 0:nn], xT[:, k, t * P:(t + 1) * P],
                                     wv_n[:, k, 0:nn], start=(k == 0), stop=(k == 11))
                nc.vector.tensor_copy(v_sb[:, t, n0:n0 + nn], mm_ps[:, 0:nn])

        # v routing: local head column lh is head lh on a q-core and head
        # 6+lh on a k-core. Write BOTH halves' destination patterns
        # statically - a consumer only reads the shards of the 4 ranks of the
        # correct half for each slot, so wrong-half writes are dead data.
        for lh in range(6):
            dests = set(_head_dests(lh)) | set(_head_dests(6 + lh))
            for (d, sl) in sorted(dests):
                nc.sync.dma_start(
                    out=send_v[d].rearrange("(a p) v -> p a v", p=P)
                    [:, :, sl * P:(sl + 1) * P],
                    in_=v_sb[:, :, lh * P:(lh + 1) * P])

    # ---------------- A2A #1b: v (overlaps with the score pass) ----------
    nc.gpsimd.collective_compute(
        "AllToAll", ALU.bypass, replica_groups=[list(range(N_CORES))],
        ins=[send_v.opt()], outs=[recv_v.opt()])

    r_v2d = recv_v.rearrange("r t v -> (r t) v")

    # ---------------- Phase D: attention, 2 head slots ----------------
    with tc.tile_pool(name="attn", bufs=1) as attn, \
         tc.tile_pool(name="aev", bufs=2) as aev:

        qT = attn.tile([P, 2, NTL * P], BF)
        kT = attn.tile([P, 2, NTL * P], BF)
        Vc = attn.tile([P, 2, NTL, P], BF)
        PT = attn.tile([P, 2, PT_COLS], BF)      # all exp'd P^T tiles (10MB)

        for slot in range(2):
            for r in range(4):
                tb = 4 * r * P
                nl = QTILES[r] * P
                nc.sync.dma_start(
                    out=qT[:, slot, tb:tb + nl],
                    in_=recv_qk[r, slot * P:(slot + 1) * P, 0:nl])
                nc.sync.dma_start(
                    out=kT[:, slot, tb:tb + nl],
                    in_=recv_qk[r + 4, slot * P:(slot + 1) * P, 0:nl])

        # V loads (gpsimd queue: these sit after A2A-v, before A2A-o0)
        with nc.gpsimd.register("vr") as rr:
            for slot in range(2):
                for r in range(4):
                    idx = slot * 4 + r
                    nc.gpsimd.reg_load(rr, tab_sb[0:1, idx:idx + 1])
                    vrow = nc.gpsimd.snap(rr)
                    nc.gpsimd.dma_start(
                        out=Vc[:, slot, 4 * r:4 * r + QTILES[r], :],
                        in_=r_v2d[ds(vrow, QTILES[r] * P),
                                  slot * P:(slot + 1) * P]
                        .rearrange("(a p) d -> p a d", p=P))

        # Pass 1: scores + exp for both slots (only needs q/k)
        with tc.tile_pool(name="psS", bufs=3, space="PSUM") as psS:
            for slot in range(2):
                for (t0, t1) in GROUPS:
                    for kt in range(t1 + 1):
                        col, c0, n = PT_OFFS[(t0, kt)]
                        sT_ps = psS.tile([P, 640], F32, tag="sT")
                        for (ja, jn) in _chunks(n):
                            nc.tensor.matmul(sT_ps[:, ja:ja + jn],
                                             kT[:, slot, kt * P:(kt + 1) * P],
                                             qT[:, slot, c0 * P + ja:c0 * P + ja + jn],
                                             start=True, stop=True)
                        pt = PT[:, slot, col:col + n]
                        nc.scalar.activation(pt, sT_ps[:, 0:n], AF.Exp,
                                             scale=SCALE)
                        if kt == 16:
                            # zero pad-key rows 32..128 (a base partition of
                            # 32 may span at most 32 rows)
                            nc.vector.tensor_scalar_mul(pt[32:64, :], pt[32:64, :], 0.0)
                            nc.vector.tensor_scalar_mul(pt[64:P, :], pt[64:P, :], 0.0)

        # Pass 2: PV + softmax denominator + normalize, per slot; each slot's
        # output is exchanged while the next chunk of compute runs.
        with tc.tile_pool(name="psO", bufs=2, space="PSUM") as psO, \
             tc.tile_pool(name="psL", bufs=1, space="PSUM") as psL, \
             tc.tile_pool(name="psR", bufs=1, space="PSUM") as psR:
            for slot in range(2):
                send_o = send_o0 if slot == 0 else send_o1
                for (t0, t1) in GROUPS:
                    ng = (t1 - t0 + 1) * P
                    oT_ps = psO.tile([P, 640], F32, tag="oT")
                    l_ps = psL.tile([1, 640], F32, tag="l")
                    for kt in range(t1 + 1):
                        col, c0, n = PT_OFFS[(t0, kt)]
                        off = (c0 - t0) * P
                        pt = PT[:, slot, col:col + n]
                        # accumulation groups are per PSUM bank: a bank's last
                        # write happens at kt == its highest column tile
                        for (ja, jn) in _bank_chunks(off, n):
                            bank = ja // 512
                            fin = (kt == min(t1, t0 + 4 * bank + 3))
                            nc.tensor.matmul(oT_ps[:, ja:ja + jn],
                                             Vc[:, slot, kt, :],
                                             pt[:, ja - off:ja - off + jn],
                                             start=(kt == 0), stop=fin)
                            nc.tensor.matmul(l_ps[:, ja:ja + jn],
                                             ones_bf, pt[:, ja - off:ja - off + jn],
                                             start=(kt == 0), stop=fin)
                    # 1/l, broadcast across partitions via ones-matmul
                    rl = aev.tile([1, 640], F32, tag="rl")
                    nc.vector.reciprocal(rl[:, 0:ng], l_ps[:, 0:ng])
                    rl_bf = aev.tile([1, 640], BF, tag="rl_bf")
                    nc.vector.tensor_copy(rl_bf[:, 0:ng], rl[:, 0:ng])
                    rl_ps = psR.tile([P, 640], F32, tag="rl_ps")
                    for (ja, jn) in _chunks(ng):
                        nc.tensor.matmul(rl_ps[:, ja:ja + jn], ones_row,
                                         rl_bf[:, ja:ja + jn],
                                         start=True, stop=True)
                    rl_bc = aev.tile([P, 640], BF, tag="rl_bc")
                    nc.vector.tensor_copy(rl_bc[:, 0:ng], rl_ps[:, 0:ng])
                    oT_sb = aev.tile([P, 640], BF, tag="oT_sb")
                    nc.vector.tensor_tensor(oT_sb[:, 0:ng], oT_ps[:, 0:ng],
                                            rl_bc[:, 0:ng], ALU.mult)
                    # slice into destination token blocks
                    g0 = t0 * P
                    for j in range(4):
                        a = max(g0, j * TOK_BLK)
                        b = min(g0 + ng, (j + 1) * TOK_BLK)
                        if a >= b:
                            continue
                        for dd in (j, j + 4):
                            nc.sync.dma_start(
                                out=send_o[dd, :, a - j * TOK_BLK:b - j * TOK_BLK],
                                in_=oT_sb[:, a - g0:b - g0])
                # exchange this slot's output
                nc.gpsimd.collective_compute(
                    "AllToAll", ALU.bypass,
                    replica_groups=[list(range(N_CORES))],
                    ins=[(send_o0 if slot == 0 else send_o1).opt()],
                    outs=[(recv_o0 if slot == 0 else recv_o1).opt()])

    def head_src(h):
        return (h, 0) if h < 8 else (h - 8, 1)

    # ---------------- Phase E: output projection ----------------
    with tc.tile_pool(name="oproj", bufs=1) as op, \
         tc.tile_pool(name="owork", bufs=3) as ow, \
         tc.tile_pool(name="psP", bufs=2, space="PSUM") as psP:

        oT_asm = op.tile([P, NH, TOK_BLK], BF)
        for h in range(NH):
            rk, sl = head_src(h)
            nc.sync.dma_start(out=oT_asm[:, h, :],
                              in_=(recv_o0 if sl == 0 else recv_o1)[rk, :, :])

        for m in range(6):
            ps = psP.tile([P, TOK_BLK], F32, tag="psP")
            for (ja, jn) in _chunks(TOK_BLK):
                for k in range(NH):
                    nc.tensor.matmul(ps[:, ja:ja + jn],
                                     wo_sb[:, k, m * P:(m + 1) * P],
                                     oT_asm[:, k, ja:ja + jn],
                                     start=(k == 0), stop=(k == NH - 1))
            oev = ow.tile([P, TOK_BLK], F32, tag="oev")
            nc.vector.tensor_copy(oev, ps)
            nc.sync.dma_start(out=outT[m * P:(m + 1) * P, :], in_=oev)


# ======================= host side =======================

def _expected_mask():
    blk = np.arange(SPAD) // P
    return (blk[:, None] >= blk[None, :]) & (np.arange(SPAD)[None, :] < S)


def _host_prep(x, freqs, wq, wk, wv, wo):
    """Build the 8 per-core input maps (bf16 activations/weights)."""
    from ml_dtypes import bfloat16

    x_pad = np.zeros((SPAD, DIM), np.float32)
    x_pad[:S] = x[0]

    # rope angle table (pure gather from freqs) -> cos/sin in fp32
    t = np.arange(S)
    fi = t // (H_ * W_)
    hi = (t % (H_ * W_)) // W_
    wi = t % W_
    ang = np.zeros((SPAD, 64), np.float64)
    ang[:S, 0:22] = freqs[fi, 0:22]
    ang[:S, 22:43] = freqs[hi, 22:43]
    ang[:S, 43:64] = freqs[wi, 43:64]

    in_maps = []
    for c in range(N_CORES):
        qr = c % 4
        x_my = np.zeros((TQ, DIM), np.float32)
        x_my[:QLEN[qr]] = x_pad[QSTART[qr]:QSTART[qr] + QLEN[qr]]
        ang_q = ang[QSTART[qr]:QSTART[qr] + QLEN[qr]]
        cs_my = np.zeros((TQ, 128), np.float32)
        cs_my[:QLEN[qr], 0:64] = np.cos(ang_q)
        cs_my[:QLEN[qr], 64:128] = np.sin(ang_q)

        tabs = np.zeros((1, 32), np.int32)
        # consumer: v source rank row offsets per slot, per quarter
        for slot in range(2):
            head = _slot_head(c, slot)
            vbase = 0 if head < 6 else 4
            for r in range(4):
                tabs[0, slot * 4 + r] = (vbase + r) * TQ

        in_maps.append({
            "x_my": x_my.astype(bfloat16),
            "w_proj": np.ascontiguousarray(wq if c < 4 else wk).astype(bfloat16),
            "wv_half": np.ascontiguousarray(
                wv[:, :VHALF] if c < 4 else wv[:, VHALF:]).astype(bfloat16),
            "wo_slice": np.ascontiguousarray(
                wo[:, (c // 4) * NHALF:(c // 4 + 1) * NHALF]).astype(bfloat16),
            "cs_in": cs_my,
            "tabs": tabs,
        })
    return in_maps


def get_nc():
    global _CACHED_NC
    if _CACHED_NC is None:
        _CACHED_NC = build_nc()
    return _CACHED_NC


def kernel(x, freqs, wq, bq, wk, bk, wv, bv, wo, bo, gq, gk,
           seq_lens, grid_sizes, mask, _run=None):
    x = np.asarray(x, np.float32)
    freqs = np.asarray(freqs, np.float32)
    wq, wk, wv, wo = (np.asarray(w, np.float32) for w in (wq, wk, wv, wo))

    assert x.shape == (1, S, DIM)
    assert int(np.asarray(seq_lens)[0]) == S, "teacher-forcing path not implemented"
    assert tuple(np.asarray(grid_sizes)[0]) == (F_, H_, W_)
    for b in (bq, bk, bv, bo):
        assert not np.any(np.asarray(b)), "nonzero bias not supported"
    for g in (gq, gk):
        assert np.all(np.asarray(g) == 1.0), "non-unit norm gain not supported"
    assert np.array_equal(np.asarray(mask), _expected_mask()), "unexpected mask"

    in_maps = _host_prep(x, freqs, wq, wk, wv, wo)

    if _run is None:
        from concourse.bass_utils import run_bass_kernel_spmd
        nc = get_nc()
        res = run_bass_kernel_spmd(nc, in_maps, list(range(N_CORES)))
        outs = [res.results[c]["outT"] for c in range(N_CORES)]
    else:
        outs = _run(in_maps)

    out = np.zeros((SPAD, DIM), np.float32)
    for c in range(N_CORES):
        r0 = (c % 4) * TOK_BLK
        n0 = (c // 4) * NHALF
        out[r0:r0 + TOK_BLK, n0:n0 + NHALF] = outs[c].T
    return out[:S][None]


# revision 9
# speedup vs baseline: 1.6445x; 1.2304x over previous
"""Trainium2 Bass kernel for CausalWanSelfAttention (block-causal attention with
RMSNorm'd+RoPE'd q/k), distributed over 8 NeuronCores via SPMD.

v3: all-bf16 data paths (fp32 PSUM accumulation), five small AllToAlls
pipelined against compute (qk-slot0, qk-slot1, v, o-slot0, o-slot1), wide
fused exp spans on the scalar engine, softmax denominator from a
DVE-accumulated sum of P^T tiles (single ones-matmul per group instead of one
per key tile), per-slot batched normalization, and a two-round output
projection (heads 0-7 accumulate to partials while the slot-1 exchange is in
flight, heads 8-11 added after).

Sharding:
  - Token quarters (tiles 4/4/4/5 of 128): cores 0-3 compute the q projection
    (full 1536 dims) for their quarter, cores 4-7 the k projection. Every core
    also computes a [quarter x 768] block of v. RMSNorm in fp32, rope in bf16.
  - Attention: 16 head-slots (2 full causal triangles per core; the 4
    duplicate slots on cores 4-7 are discarded). Scores are computed
    transposed (sT[keys,q] = kT.T @ qT) so P^T feeds the V-matmul directly.
  - Each core runs the output projection for a [544-token x 768-out-dim]
    block; the host assembles the blocks.
"""
import math
import sys

sys.path.insert(0, "/opt/trn_rl_repo")

import numpy as np

import concourse.bass as bass
import concourse.tile as tile
from concourse import bacc, mybir
from concourse.bass import ds
from concourse.masks import make_identity

F32 = mybir.dt.float32
BF = mybir.dt.bfloat16
I32 = mybir.dt.int32
AF = mybir.ActivationFunctionType
ALU = mybir.AluOpType

# problem constants (hardcoded per contract)
P = 128
DIM = 1536
NH = 12
HD = 128
S = 2080
SPAD = 2176
NTL = 17
F_, H_, W_ = 4, 20, 26
EPS = 1e-6
N_CORES = 8

QTILES = [4, 4, 4, 5]          # token tiles per quarter
QSTART = [0, 512, 1024, 1536]  # token start per quarter
QLEN = [512, 512, 512, 640]
TQ = 640                       # uniform (padded) tokens per core
NT = 5                         # uniform token tiles per core
VHALF = 768

TOK_BLK = 544                  # O-proj tokens per core
NHALF = 768                    # O-proj out-dims per core
GROUPS = [(0, 3), (4, 7), (8, 11), (12, 16)]
SCALE = 1.0 / math.sqrt(HD)

_CACHED_NC = None


def _slot_head(c, slot):
    return c if slot == 0 else 8 + (c % 4)


def _head_dests(h):
    """Destination (core, slot) pairs that attend head h."""
    if h < 8:
        return [(h, 0)]
    return [(h - 8, 1), (h - 4, 1)]


def _chunks(total, step=512):
    out = []
    a = 0
    while a < total:
        out.append((a, min(step, total - a)))
        a += step
    return out


def _bank_chunks(off, n, bank=512):
    """Split [off, off+n) at absolute multiples of `bank` (PSUM bank size)."""
    out = []
    a = off
    end = off + n
    while a < end:
        b = min(end, (a // bank + 1) * bank)
        out.append((a, b - a))
        a = b
    return out


def _pt_offsets():
    """Column offset of each (group, kt) P^T tile in the per-slot PT store."""
    offs = {}
    col = 0
    for (t0, t1) in GROUPS:
        for kt in range(t1 + 1):
            c0 = max(t0, kt)
            n = (t1 - c0 + 1) * P
            offs[(t0, kt)] = (col, c0, n)
            col += n
    return offs, col


PT_OFFS, PT_COLS = _pt_offsets()   # PT_COLS = 19584


def _exp_spans():
    """Greedily pack consecutive (group, kt) score tiles into <=2048-col PSUM
    spans; one exp instruction per span. Returns a list of spans, each a list
    of (t0, kt, span_off, n)."""
    spans = []
    cur, cur_len = [], 0
    for (t0, t1) in GROUPS:
        for kt in range(t1 + 1):
            col, c0, n = PT_OFFS[(t0, kt)]
            if cur_len + n > 2048:
                spans.append(cur)
                cur, cur_len = [], 0
            cur.append((t0, kt, cur_len, n))
            cur_len += n
    if cur:
        spans.append(cur)
    return spans


EXP_SPANS = _exp_spans()


def build_nc():
    nc = bacc.Bacc("TRN2", target_bir_lowering=False, debug=False,
                   num_devices=N_CORES)

    x_my = nc.dram_tensor("x_my", [TQ, DIM], BF, kind="ExternalInput").ap()
    w_proj = nc.dram_tensor("w_proj", [DIM, DIM], BF, kind="ExternalInput").ap()
    wv_half = nc.dram_tensor("wv_half", [DIM, VHALF], BF, kind="ExternalInput").ap()
    wo_slice = nc.dram_tensor("wo_slice", [DIM, NHALF], BF, kind="ExternalInput").ap()
    cs_in = nc.dram_tensor("cs_in", [TQ, 128], BF, kind="ExternalInput").ap()
    tabs = nc.dram_tensor("tabs", [1, 32], I32, kind="ExternalInput").ap()
    outT = nc.dram_tensor("outT", [NHALF, TOK_BLK], F32, kind="ExternalOutput").ap()

    with tile.TileContext(nc) as tc:
        _body(tc, x_my, w_proj, wv_half, wo_slice, cs_in, tabs, outT)
    nc.compile()
    return nc


def _body(tc, *args):
    from contextlib import ExitStack
    with ExitStack() as es:
        const = es.enter_context(tc.tile_pool(name="const", bufs=1))
        dram = es.enter_context(tc.tile_pool(name="dram", bufs=1, space="DRAM"))
        _body2(tc, const, dram, *args)


def _body2(tc, const, dram,
           x_my, w_proj, wv_half, wo_slice, cs_in, tabs, outT):
    nc = tc.nc

    ident = const.tile([P, P], BF)
    make_identity(nc, ident)
    ones_bf = const.tile([P, 1], BF)
    nc.vector.memset(ones_bf, 1.0)
    ones_row = const.tile([1, P], BF)
    nc.vector.memset(ones_row, 1.0)
    eps_t = const.tile([P, 1], F32)
    nc.vector.memset(eps_t, EPS)
    tab_sb = const.tile([1, 32], I32)
    nc.sync.dma_start(out=tab_sb, in_=tabs)
    wo_sb = const.tile([P, 12, NHALF], BF)
    nc.sync.dma_start(out=wo_sb,
                      in_=wo_slice.rearrange("(k p) d -> p k d", p=P))

    # rope tables: cs_in = [cos | sin] per token, bf16
    cs_sb = const.tile([P, NT, 128], BF)
    nc.sync.dma_start(out=cs_sb, in_=cs_in.rearrange("(a p) c -> p a c", p=P))
    cos_sb = cs_sb[:, :, 0:64]
    sin_sb = cs_sb[:, :, 64:128]

    # collective buffers (bf16)
    send_qk0 = dram.tile([N_CORES, P, TQ], BF)
    recv_qk0 = dram.tile([N_CORES, P, TQ], BF)
    send_qk1 = dram.tile([N_CORES, P, TQ], BF)
    recv_qk1 = dram.tile([N_CORES, P, TQ], BF)
    send_v = dram.tile([N_CORES, TQ, 256], BF)
    recv_v = dram.tile([N_CORES, TQ, 256], BF)
    send_o0 = dram.tile([N_CORES, P, TOK_BLK], BF)
    recv_o0 = dram.tile([N_CORES, P, TOK_BLK], BF)
    send_o1 = dram.tile([N_CORES, P, TOK_BLK], BF)
    recv_o1 = dram.tile([N_CORES, P, TOK_BLK], BF)
    send_qk = [send_qk0, send_qk1]
    recv_qk = [recv_qk0, recv_qk1]

    # ---------------- Phase A+B: xT, q/k projection + RMS + rope ----------------
    with tc.tile_pool(name="resident", bufs=1) as res, \
         tc.tile_pool(name="xtiles", bufs=2) as xtiles, \
         tc.tile_pool(name="wpool", bufs=2) as wpool, \
         tc.tile_pool(name="work", bufs=4) as work, \
         tc.tile_pool(name="evict", bufs=4) as evict, \
         tc.tile_pool(name="psA", bufs=3, space="PSUM") as psA, \
         tc.tile_pool(name="psT", bufs=2, space="PSUM") as psT:

        xT = res.tile([P, 12, TQ], BF)            # x^T, d-major (2.0MB)
        q_raw = res.tile([P, NT, DIM], BF)        # projection out, token-major

        # A: load x tiles, PE-transpose into xT (4 transposes packed per
        # PSUM tile so one DVE copy evicts 512 columns)
        for t in range(NT):
            x_t = xtiles.tile([P, DIM], BF, tag="x_t")
            nc.sync.dma_start(out=x_t, in_=x_my[t * P:(t + 1) * P, :])
            for a in range(3):
                tp = psT.tile([P, 512], BF, tag="tpx")
                for b in range(4):
                    k = 4 * a + b
                    nc.tensor.transpose(tp[:, b * P:(b + 1) * P],
                                        x_t[:, k * P:(k + 1) * P], ident)
                nc.vector.tensor_copy(
                    xT[:, 4 * a:4 * a + 4, t * P:(t + 1) * P],
                    tp.rearrange("p (b c) -> p b c", b=4))

        # B: q (or k) projection, n-chunk outer so weights stream once.
        # The fp32 PSUM result is squared+accumulated on ACT (for the RMS
        # denominator) and also copy-cast to bf16 q_raw on ACT.
        ssq = work.tile([P, NT, 3], F32, tag="ssq", bufs=1)
        for n in range(3):
            w_n = wpool.tile([P, 12, 512], BF, tag="w_n")
            nc.sync.dma_start(
                out=w_n,
                in_=w_proj[:, n * 512:(n + 1) * 512]
                .rearrange("(k p) d -> p k d", p=P))
            for t in range(NT):
                mm_ps = psA.tile([P, 512], F32, tag="mm")
                for k in range(12):
                    nc.tensor.matmul(mm_ps, xT[:, k, t * P:(t + 1) * P],
                                     w_n[:, k, :], start=(k == 0), stop=(k == 11))
                sq_scr = work.tile([P, 512], F32, tag="sq_scr")
                nc.scalar.activation(sq_scr, mm_ps, AF.Square,
                                     accum_out=ssq[:, t, n:n + 1])
                nc.scalar.activation(q_raw[:, t, n * 512:(n + 1) * 512],
                                     mm_ps, AF.Copy)

        # RMS + rope per token tile (in-place on q_raw, bf16 at 2x DVE rate)
        for t in range(NT):
            s01 = work.tile([P, 1], F32, tag="s01")
            nc.vector.tensor_tensor(s01, ssq[:, t, 0:1], ssq[:, t, 1:2], ALU.add)
            stot = work.tile([P, 1], F32, tag="stot")
            nc.vector.tensor_tensor(stot, s01, ssq[:, t, 2:3], ALU.add)
            sq_t = work.tile([P, 1], F32, tag="sq_t")
            nc.scalar.activation(sq_t, stot, AF.Sqrt, bias=eps_t,
                                 scale=1.0 / DIM)
            rsq = work.tile([P, 1], F32, tag="rsq")
            nc.vector.reciprocal(rsq, sq_t)
            crsq = work.tile([P, 64], BF, tag="crsq")
            srsq = work.tile([P, 64], BF, tag="srsq")
            nc.vector.tensor_scalar_mul(crsq, cos_sb[:, t, :], rsq)
            nc.vector.tensor_scalar_mul(srsq, sin_sb[:, t, :], rsq)
            cb = bass.AP(tensor=crsq.tensor, offset=crsq.offset,
                         ap=[crsq.ap[0], [0, NH], crsq.ap[1]])
            sbb = bass.AP(tensor=srsq.tensor, offset=srsq.offset,
                          ap=[srsq.ap[0], [0, NH], srsq.ap[1]])
            qh = q_raw[:, t, :].rearrange("p (h c two) -> p h c two", h=NH, two=2)
            qe = qh[:, :, :, 0]
            qo = qh[:, :, :, 1]
            tA = work.tile([P, NH, 64], BF, tag="tA")
            tB = work.tile([P, NH, 64], BF, tag="tB")
            tC = work.tile([P, NH, 64], BF, tag="tC")
            tD = work.tile([P, NH, 64], BF, tag="tD")
            nc.vector.tensor_tensor(tA, qe, cb, ALU.mult)
            nc.vector.tensor_tensor(tB, qo, sbb, ALU.mult)
            nc.vector.tensor_tensor(tC, qe, sbb, ALU.mult)
            nc.vector.tensor_tensor(tD, qo, cb, ALU.mult)
            nc.vector.tensor_tensor(qe, tA, tB, ALU.subtract)
            nc.vector.tensor_tensor(qo, tC, tD, ALU.add)

        # transpose roped q/k into a per-head bf16 stage (transposes packed
        # 4-wide in PSUM), then DMA per route into the per-slot send buffers
        for h in range(12):
            stage_h = evict.tile([P, TQ], BF, tag="stage_h", bufs=3)
            tp5 = psT.tile([P, TQ], BF, tag="tpq")
            for t in range(NT):
                nc.tensor.transpose(tp5[:, t * P:(t + 1) * P],
                                    q_raw[:, t, h * P:(h + 1) * P], ident)
            nc.vector.tensor_copy(stage_h, tp5)
            for (d, sl) in _head_dests(h):
                nc.sync.dma_start(out=send_qk[sl][d, :, :], in_=stage_h)

        # ---------------- A2A: qk slot 0, then slot 1 ----------------
        nc.gpsimd.collective_compute(
            "AllToAll", ALU.bypass, replica_groups=[list(range(N_CORES))],
            ins=[send_qk0.opt()], outs=[recv_qk0.opt()])
        nc.gpsimd.collective_compute(
            "AllToAll", ALU.bypass, replica_groups=[list(range(N_CORES))],
            ins=[send_qk1.opt()], outs=[recv_qk1.opt()])

        # C: v projection [TQ, VHALF] into v_sb (evicted via ACT copy-cast),
        # then route columns per dest on the vector queue
        v_sb = res.tile([P, NT, VHALF], BF)
        for n0, nn in ((0, 512), (512, 256)):
            wv_n = wpool.tile([P, 12, 512], BF, tag="w_n")
            nc.sync.dma_start(
                out=wv_n[:, :, 0:nn],
                in_=wv_half[:, n0:n0 + nn]
                .rearrange("(k p) d -> p k d", p=P))
            for t in range(NT):
                mm_ps = psA.tile([P, 512], F32, tag="mm")
                for k in range(12):
                    nc.tensor.matmul(mm_ps[:, 0:nn], xT[:, k, t * P:(t + 1) * P],
                                     wv_n[:, k, 0:nn], start=(k == 0), stop=(k == 11))
                nc.scalar.activation(v_sb[:, t, n0:n0 + nn], mm_ps[:, 0:nn],
                                     AF.Copy)

        # v routing: local head column lh is head lh on a q-core and head
        # 6+lh on a k-core. Write BOTH halves' destination patterns
        # statically - a consumer only reads the shards of the 4 ranks of the
        # correct half for each slot, so wrong-half writes are dead data.
        for lh in range(6):
            dests = set(_head_dests(lh)) | set(_head_dests(6 + lh))
            for (d, sl) in sorted(dests):
                nc.scalar.dma_start(
                    out=send_v[d].rearrange("(a p) v -> p a v", p=P)
                    [:, :, sl * P:(sl + 1) * P],
                    in_=v_sb[:, :, lh * P:(lh + 1) * P])

    # ---------------- A2A: v (overlaps with the score pass) ----------
    nc.gpsimd.collective_compute(
        "AllToAll", ALU.bypass, replica_groups=[list(range(N_CORES))],
        ins=[send_v.opt()], outs=[recv_v.opt()])

    r_v2d = recv_v.rearrange("r t v -> (r t) v")

    # ---------------- Phase D: attention, 2 head slots ----------------
    with tc.tile_pool(name="attn", bufs=1) as attn, \
         tc.tile_pool(name="aev", bufs=2) as aev:

        qT = attn.tile([P, 2, NTL * P], BF)
        kT = attn.tile([P, 2, NTL * P], BF)
        Vc = attn.tile([P, 2, NTL, P], BF)
        PT = attn.tile([P, 2, PT_COLS], BF)      # all exp'd P^T tiles (10MB)
        PTS = attn.tile([P, 2, 4, 640], BF)      # per-group sum over kt of P^T
        oT_slot = attn.tile([P, 2, NTL * P], BF)  # unnormalized outputs
        l_slot = attn.tile([1, 2, NTL * P], F32)

        for slot in range(2):
            for r in range(4):
                tb = 4 * r * P
                nl = QTILES[r] * P
                nc.sync.dma_start(out=qT[:, slot, tb:tb + nl],
                                  in_=recv_qk[slot][r, :, 0:nl])
                nc.sync.dma_start(out=kT[:, slot, tb:tb + nl],
                                  in_=recv_qk[slot][r + 4, :, 0:nl])

        # V loads (gpsimd queue: these sit after A2A-v, before A2A-o0)
        with nc.gpsimd.register("vr") as rr:
            for slot in range(2):
                for r in range(4):
                    idx = slot * 4 + r
                    nc.gpsimd.reg_load(rr, tab_sb[0:1, idx:idx + 1])
                    vrow = nc.gpsimd.snap(rr)
                    nc.gpsimd.dma_start(
                        out=Vc[:, slot, 4 * r:4 * r + QTILES[r], :],
                        in_=r_v2d[ds(vrow, QTILES[r] * P),
                                  slot * P:(slot + 1) * P]
                        .rearrange("(a p) d -> p a d", p=P))

        # Pass 1: scores + fused-span exp for both slots (needs only q/k);
        # DVE accumulates each group's P^T tiles into PTS for the softmax
        # denominator.
        with tc.tile_pool(name="psS", bufs=2, space="PSUM") as psS:
            for slot in range(2):
                for span in EXP_SPANS:
                    span_len = sum(e[3] for e in span)
                    sp = psS.tile([P, 2048], F32, tag="sp")
                    for (t0, kt, so, n) in span:
                        col, c0, _ = PT_OFFS[(t0, kt)]
                        for (ja, jn) in _bank_chunks(so, n):
                            qa = c0 * P + (ja - so)
                            nc.tensor.matmul(sp[:, ja:ja + jn],
                                             kT[:, slot, kt * P:(kt + 1) * P],
                                             qT[:, slot, qa:qa + jn],
                                             start=True, stop=True)
                    col0 = PT_OFFS[(span[0][0], span[0][1])][0]
                    nc.scalar.activation(PT[:, slot, col0:col0 + span_len],
                                         sp[:, 0:span_len], AF.Exp, scale=SCALE)
                    for (t0, kt, so, n) in span:
                        col, c0, _ = PT_OFFS[(t0, kt)]
                        pt = PT[:, slot, col:col + n]
                        if kt == 16:
                            # zero pad-key rows 32..128 (a base partition of
                            # 32 may span at most 32 rows)
                            nc.vector.tensor_scalar_mul(pt[32:64, :],
                                                        pt[32:64, :], 0.0)
                            nc.vector.tensor_scalar_mul(pt[64:P, :],
                                                        pt[64:P, :], 0.0)
                        # accumulate into the group sum (kt==0 initializes)
                        gi = GROUPS.index((t0, {0: 3, 4: 7, 8: 11, 12: 16}[t0]))
                        off = (c0 - t0) * P
                        dst = PTS[:, slot, gi, off:off + n]
                        if kt == 0:
                            nc.vector.tensor_copy(dst, pt)
                        else:
                            nc.vector.tensor_tensor(dst, dst, pt, ALU.add)

        # Pass 2: PV per slot; denominator from one ones-matmul over PTS;
        # per-slot batched normalization; each slot's output is exchanged
        # while the next chunk of compute runs.
        with tc.tile_pool(name="psO", bufs=2, space="PSUM") as psO, \
             tc.tile_pool(name="psL", bufs=1, space="PSUM") as psL, \
             tc.tile_pool(name="psR", bufs=2, space="PSUM") as psR:
            for slot in range(2):
                send_o = send_o0 if slot == 0 else send_o1
                for gi, (t0, t1) in enumerate(GROUPS):
                    ng = (t1 - t0 + 1) * P
                    g0 = t0 * P
                    oT_ps = psO.tile([P, 640], F32, tag="oT")
                    for kt in range(t1 + 1):
                        col, c0, n = PT_OFFS[(t0, kt)]
                        off = (c0 - t0) * P
                        pt = PT[:, slot, col:col + n]
                        # accumulation groups are per PSUM bank: a bank's last
                        # write happens at kt == its highest column tile
                        for (ja, jn) in _bank_chunks(off, n):
                            bank = ja // 512
                            fin = (kt == min(t1, t0 + 4 * bank + 3))
                            nc.tensor.matmul(oT_ps[:, ja:ja + jn],
                                             Vc[:, slot, kt, :],
                                             pt[:, ja - off:ja - off + jn],
                                             start=(kt == 0), stop=fin)
                    l_ps = psL.tile([1, 640], F32, tag="l")
                    for (ja, jn) in _chunks(ng):
                        nc.tensor.matmul(l_ps[:, ja:ja + jn], ones_bf,
                                         PTS[:, slot, gi, ja:ja + jn],
                                         start=True, stop=True)
                    nc.vector.tensor_copy(l_slot[:, slot, g0:g0 + ng],
                                          l_ps[:, 0:ng])
                    nc.vector.tensor_copy(oT_slot[:, slot, g0:g0 + ng],
                                          oT_ps[:, 0:ng])
                # batched normalization for the whole slot
                rl = aev.tile([1, NTL * P], F32, tag="rl")
                nc.vector.reciprocal(rl, l_slot[:, slot, :])
                rl_bf = aev.tile([1, NTL * P], BF, tag="rl_bf")
                nc.vector.tensor_copy(rl_bf, rl)
                rl_bc = aev.tile([P, NTL * P], BF, tag="rl_bc")
                for (ja, jn) in _chunks(NTL * P):
                    rl_ps = psR.tile([P, 512], F32, tag="rl_ps")
                    nc.tensor.matmul(rl_ps[:, 0:jn], ones_row,
                                     rl_bf[:, ja:ja + jn],
                                     start=True, stop=True)
                    nc.vector.tensor_copy(rl_bc[:, ja:ja + jn], rl_ps[:, 0:jn])
                oT_n = aev.tile([P, NTL * P], BF, tag="oT_n")
                nc.vector.tensor_tensor(oT_n, oT_slot[:, slot, :], rl_bc,
                                        ALU.mult)
                # slice into destination token blocks
                for j in range(4):
                    for dd in (j, j + 4):
                        nc.sync.dma_start(
                            out=send_o[dd, :, :],
                            in_=oT_n[:, j * TOK_BLK:(j + 1) * TOK_BLK])
                # exchange this slot's output
                nc.gpsimd.collective_compute(
                    "AllToAll", ALU.bypass,
                    replica_groups=[list(range(N_CORES))],
                    ins=[(send_o0 if slot == 0 else send_o1).opt()],
                    outs=[(recv_o0 if slot == 0 else recv_o1).opt()])

    def head_src(h):
        return (h, 0) if h < 8 else (h - 8, 1)

    # ---------------- Phase E: output projection ----------------
    # Round 1 (heads 0-7, slot-0 data) accumulates to fp32 partials in SBUF
    # while the slot-1 exchange is still in flight; round 2 adds heads 8-11.
    with tc.tile_pool(name="oproj", bufs=1) as op, \
         tc.tile_pool(name="owork", bufs=3) as ow, \
         tc.tile_pool(name="psP", bufs=2, space="PSUM") as psP:

        oT_asm = op.tile([P, NH, TOK_BLK], BF)
        for h in range(NH):
            rk, sl = head_src(h)
            nc.sync.dma_start(out=oT_asm[:, h, :],
                              in_=(recv_o0 if sl == 0 else recv_o1)[rk, :, :])

        part = op.tile([P, 6, TOK_BLK], F32)
        for m in range(6):
            ps = psP.tile([P, TOK_BLK], F32, tag="psP")
            for (ja, jn) in _chunks(TOK_BLK):
                for k in range(8):
                    nc.tensor.matmul(ps[:, ja:ja + jn],
                                     wo_sb[:, k, m * P:(m + 1) * P],
                                     oT_asm[:, k, ja:ja + jn],
                                     start=(k == 0), stop=(k == 7))
            nc.vector.tensor_copy(part[:, m, :], ps)
        for m in range(6):
            ps = psP.tile([P, TOK_BLK], F32, tag="psP")
            for (ja, jn) in _chunks(TOK_BLK):
                for k in range(8, NH):
                    nc.tensor.matmul(ps[:, ja:ja + jn],
                                     wo_sb[:, k, m * P:(m + 1) * P],
                                     oT_asm[:, k, ja:ja + jn],
                                     start=(k == 8), stop=(k == NH - 1))
            oev = ow.tile([P, TOK_BLK], F32, tag="oev")
            nc.vector.tensor_tensor(oev, part[:, m, :], ps, ALU.add)
            nc.sync.dma_start(out=outT[m * P:(m + 1) * P, :], in_=oev)


# ======================= host side =======================

def _expected_mask():
    blk = np.arange(SPAD) // P
    return (blk[:, None] >= blk[None, :]) & (np.arange(SPAD)[None, :] < S)


def _host_prep(x, freqs, wq, wk, wv, wo):
    """Build the 8 per-core input maps (bf16 activations/weights)."""
    from ml_dtypes import bfloat16

    x_pad = np.zeros((SPAD, DIM), np.float32)
    x_pad[:S] = x[0]

    # rope angle table (pure gather from freqs) -> cos/sin
    t = np.arange(S)
    fi = t // (H_ * W_)
    hi = (t % (H_ * W_)) // W_
    wi = t % W_
    ang = np.zeros((SPAD, 64), np.float64)
    ang[:S, 0:22] = freqs[fi, 0:22]
    ang[:S, 22:43] = freqs[hi, 22:43]
    ang[:S, 43:64] = freqs[wi, 43:64]

    in_maps = []
    for c in range(N_CORES):
        qr = c % 4
        x_my = np.zeros((TQ, DIM), np.float32)
        x_my[:QLEN[qr]] = x_pad[QSTART[qr]:QSTART[qr] + QLEN[qr]]
        ang_q = ang[QSTART[qr]:QSTART[qr] + QLEN[qr]]
        cs_my = np.zeros((TQ, 128), np.float32)
        cs_my[:QLEN[qr], 0:64] = np.cos(ang_q)
        cs_my[:QLEN[qr], 64:128] = np.sin(ang_q)

        tabs = np.zeros((1, 32), np.int32)
        # consumer: v source rank row offsets per slot, per quarter
        for slot in range(2):
            head = _slot_head(c, slot)
            vbase = 0 if head < 6 else 4
            for r in range(4):
                tabs[0, slot * 4 + r] = (vbase + r) * TQ

        in_maps.append({
            "x_my": x_my.astype(bfloat16),
            "w_proj": np.ascontiguousarray(wq if c < 4 else wk).astype(bfloat16),
            "wv_half": np.ascontiguousarray(
                wv[:, :VHALF] if c < 4 else wv[:, VHALF:]).astype(bfloat16),
            "wo_slice": np.ascontiguousarray(
                wo[:, (c // 4) * NHALF:(c // 4 + 1) * NHALF]).astype(bfloat16),
            "cs_in": cs_my.astype(bfloat16),
            "tabs": tabs,
        })
    return in_maps


def get_nc():
    global _CACHED_NC
    if _CACHED_NC is None:
        _CACHED_NC = build_nc()
    return _CACHED_NC


def kernel(x, freqs, wq, bq, wk, bk, wv, bv, wo, bo, gq, gk,
           seq_lens, grid_sizes, mask, _run=None):
    x = np.asarray(x, np.float32)
    freqs = np.asarray(freqs, np.float32)
    wq, wk, wv, wo = (np.asarray(w, np.float32) for w in (wq, wk, wv, wo))

    assert x.shape == (1, S, DIM)
    assert int(np.asarray(seq_lens)[0]) == S, "teacher-forcing path not implemented"
    assert tuple(np.asarray(grid_sizes)[0]) == (F_, H_, W_)
    for b in (bq, bk, bv, bo):
        assert not np.any(np.asarray(b)), "nonzero bias not supported"
    for g in (gq, gk):
        assert np.all(np.asarray(g) == 1.0), "non-unit norm gain not supported"
    assert np.array_equal(np.asarray(mask), _expected_mask()), "unexpected mask"

    in_maps = _host_prep(x, freqs, wq, wk, wv, wo)

    if _run is None:
        from concourse.bass_utils import run_bass_kernel_spmd
        nc = get_nc()
        res = run_bass_kernel_spmd(nc, in_maps, list(range(N_CORES)))
        outs = [res.results[c]["outT"] for c in range(N_CORES)]
    else:
        outs = _run(in_maps)

    out = np.zeros((SPAD, DIM), np.float32)
    for c in range(N_CORES):
        r0 = (c % 4) * TOK_BLK
        n0 = (c // 4) * NHALF
        out[r0:r0 + TOK_BLK, n0:n0 + NHALF] = outs[c].T
    return out[:S][None]


# revision 16
# speedup vs baseline: 1.6912x; 1.0284x over previous
"""Trainium2 Bass kernel for CausalWanSelfAttention (block-causal attention with
RMSNorm'd+RoPE'd q/k), distributed over 8 NeuronCores via SPMD.

v3: all-bf16 data paths (fp32 PSUM accumulation), five small AllToAlls
pipelined against compute (qk-slot0, qk-slot1, v, o-slot0, o-slot1), wide
fused exp spans on the scalar engine, softmax denominator from a
DVE-accumulated sum of P^T tiles (single ones-matmul per group instead of one
per key tile), per-slot batched normalization, and a two-round output
projection (heads 0-7 accumulate to partials while the slot-1 exchange is in
flight, heads 8-11 added after).

Sharding:
  - Token quarters (tiles 4/4/4/5 of 128): cores 0-3 compute the q projection
    (full 1536 dims) for their quarter, cores 4-7 the k projection. Every core
    also computes a [quarter x 768] block of v. RMSNorm in fp32, rope in bf16.
  - Attention: 16 head-slots (2 full causal triangles per core; the 4
    duplicate slots on cores 4-7 are discarded). Scores are computed
    transposed (sT[keys,q] = kT.T @ qT) so P^T feeds the V-matmul directly.
  - Each core runs the output projection for a [544-token x 768-out-dim]
    block; the host assembles the blocks.
"""
import math
import sys

sys.path.insert(0, "/opt/trn_rl_repo")

import numpy as np

import concourse.bass as bass
import concourse.tile as tile
from concourse import bacc, mybir
from concourse.bass import ds
from concourse.masks import make_identity

F32 = mybir.dt.float32
BF = mybir.dt.bfloat16
I32 = mybir.dt.int32
AF = mybir.ActivationFunctionType
ALU = mybir.AluOpType

# problem constants (hardcoded per contract)
P = 128
DIM = 1536
NH = 12
HD = 128
S = 2080
SPAD = 2176
NTL = 17
F_, H_, W_ = 4, 20, 26
EPS = 1e-6
N_CORES = 8

QTILES = [4, 4, 4, 5]          # token tiles per quarter
QSTART = [0, 512, 1024, 1536]  # token start per quarter
QLEN = [512, 512, 512, 640]
TQ = 640                       # uniform (padded) tokens per core
NT = 5                         # uniform token tiles per core
VHALF = 768

TOK_BLK = 544                  # O-proj tokens per core
NHALF = 768                    # O-proj out-dims per core
GROUPS = [(0, 3), (4, 7), (8, 11), (12, 16)]
SCALE = 1.0 / math.sqrt(HD)

_CACHED_NC = None


def _slot_head(c, slot):
    return c if slot == 0 else 8 + (c % 4)


def _head_dests(h):
    """Destination (core, slot) pairs that attend head h."""
    if h < 8:
        return [(h, 0)]
    return [(h - 8, 1), (h - 4, 1)]


def _chunks(total, step=512):
    out = []
    a = 0
    while a < total:
        out.append((a, min(step, total - a)))
        a += step
    return out


def _bank_chunks(off, n, bank=512):
    """Split [off, off+n) at absolute multiples of `bank` (PSUM bank size)."""
    out = []
    a = off
    end = off + n
    while a < end:
        b = min(end, (a // bank + 1) * bank)
        out.append((a, b - a))
        a = b
    return out


def _pt_offsets():
    """Column offset of each (group, kt) P^T tile in the per-slot PT store."""
    offs = {}
    col = 0
    for (t0, t1) in GROUPS:
        for kt in range(t1 + 1):
            c0 = max(t0, kt)
            n = (t1 - c0 + 1) * P
            offs[(t0, kt)] = (col, c0, n)
            col += n
    return offs, col


PT_OFFS, PT_COLS = _pt_offsets()   # PT_COLS = 19584


def _exp_spans():
    """Greedily pack consecutive (group, kt) score tiles into <=1536-col PSUM
    spans; one exp instruction per span. Returns a list of spans, each a list
    of (t0, kt, span_off, n)."""
    spans = []
    cur, cur_len = [], 0
    for (t0, t1) in GROUPS:
        for kt in range(t1 + 1):
            col, c0, n = PT_OFFS[(t0, kt)]
            if cur_len + n > 1536:
                spans.append(cur)
                cur, cur_len = [], 0
            cur.append((t0, kt, cur_len, n))
            cur_len += n
    if cur:
        spans.append(cur)
    return spans


EXP_SPANS = _exp_spans()


def build_nc():
    nc = bacc.Bacc("TRN2", target_bir_lowering=False, debug=False,
                   num_devices=N_CORES)

    x_my = nc.dram_tensor("x_my", [TQ, DIM], BF, kind="ExternalInput").ap()
    w_proj = nc.dram_tensor("w_proj", [DIM, DIM], BF, kind="ExternalInput").ap()
    wv_half = nc.dram_tensor("wv_half", [DIM, VHALF], BF, kind="ExternalInput").ap()
    wo_slice = nc.dram_tensor("wo_slice", [DIM, NHALF], BF, kind="ExternalInput").ap()
    cs_in = nc.dram_tensor("cs_in", [TQ, 128], BF, kind="ExternalInput").ap()
    tabs = nc.dram_tensor("tabs", [1, 32], I32, kind="ExternalInput").ap()
    outT = nc.dram_tensor("outT", [NHALF, TOK_BLK], F32, kind="ExternalOutput").ap()

    with tile.TileContext(nc) as tc:
        _body(tc, x_my, w_proj, wv_half, wo_slice, cs_in, tabs, outT)
    nc.compile()
    return nc


def _body(tc, *args):
    from contextlib import ExitStack
    with ExitStack() as es:
        const = es.enter_context(tc.tile_pool(name="const", bufs=1))
        dram = es.enter_context(tc.tile_pool(name="dram", bufs=1, space="DRAM"))
        _body2(tc, const, dram, *args)


def _body2(tc, const, dram,
           x_my, w_proj, wv_half, wo_slice, cs_in, tabs, outT):
    nc = tc.nc

    ident = const.tile([P, P], BF)
    make_identity(nc, ident)
    ones_bf = const.tile([P, 1], BF)
    nc.vector.memset(ones_bf, 1.0)
    ones_row = const.tile([1, P], BF)
    nc.vector.memset(ones_row, 1.0)
    eps_t = const.tile([P, 1], F32)
    nc.vector.memset(eps_t, EPS)
    tab_sb = const.tile([1, 32], I32)
    nc.sync.dma_start(out=tab_sb, in_=tabs)
    wo_sb = const.tile([P, 12, NHALF], BF)
    nc.sync.dma_start(out=wo_sb,
                      in_=wo_slice.rearrange("(k p) d -> p k d", p=P))

    # rope tables: cs_in = [cos | sin] per token, bf16
    cs_sb = const.tile([P, NT, 128], BF)
    nc.sync.dma_start(out=cs_sb, in_=cs_in.rearrange("(a p) c -> p a c", p=P))
    cos_sb = cs_sb[:, :, 0:64]
    sin_sb = cs_sb[:, :, 64:128]

    # collective buffers (bf16)
    send_qk0 = dram.tile([N_CORES, P, TQ], BF)
    recv_qk0 = dram.tile([N_CORES, P, TQ], BF)
    send_qk1 = dram.tile([N_CORES, P, TQ], BF)
    recv_qk1 = dram.tile([N_CORES, P, TQ], BF)
    send_v = dram.tile([N_CORES, TQ, 256], BF)
    recv_v = dram.tile([N_CORES, TQ, 256], BF)
    send_o0 = dram.tile([N_CORES, P, TOK_BLK], BF)
    recv_o0 = dram.tile([N_CORES, P, TOK_BLK], BF)
    send_o1 = dram.tile([N_CORES, P, TOK_BLK], BF)
    recv_o1 = dram.tile([N_CORES, P, TOK_BLK], BF)
    send_qk = [send_qk0, send_qk1]
    recv_qk = [recv_qk0, recv_qk1]

    # ---------------- Phase A+B: xT, q/k projection + RMS + rope ----------------
    with tc.tile_pool(name="resident", bufs=1) as res, \
         tc.tile_pool(name="xtiles", bufs=2) as xtiles, \
         tc.tile_pool(name="wpool", bufs=2) as wpool, \
         tc.tile_pool(name="work", bufs=4) as work, \
         tc.tile_pool(name="evict", bufs=4) as evict, \
         tc.tile_pool(name="psA", bufs=3, space="PSUM") as psA, \
         tc.tile_pool(name="psT", bufs=2, space="PSUM") as psT:

        xT = res.tile([P, 12, TQ], BF)            # x^T, d-major (2.0MB)
        q_raw = res.tile([P, NT, DIM], BF)        # projection out, token-major

        # A: load x tiles, PE-transpose into xT (4 transposes packed per
        # PSUM tile so one DVE copy evicts 512 columns)
        for t in range(NT):
            x_t = xtiles.tile([P, DIM], BF, tag="x_t")
            nc.sync.dma_start(out=x_t, in_=x_my[t * P:(t + 1) * P, :])
            for a in range(3):
                tp = psT.tile([P, 512], BF, tag="tpx")
                for b in range(4):
                    k = 4 * a + b
                    nc.tensor.transpose(tp[:, b * P:(b + 1) * P],
                                        x_t[:, k * P:(k + 1) * P], ident)
                nc.vector.tensor_copy(
                    xT[:, 4 * a:4 * a + 4, t * P:(t + 1) * P],
                    tp.rearrange("p (b c) -> p b c", b=4))

        # B: q (or k) projection, n-chunk outer so weights stream once.
        # The fp32 PSUM result is squared+accumulated on ACT (for the RMS
        # denominator) and also copy-cast to bf16 q_raw on ACT.
        ssq = work.tile([P, NT, 3], F32, tag="ssq", bufs=1)
        for n in range(3):
            w_n = wpool.tile([P, 12, 512], BF, tag="w_n")
            nc.sync.dma_start(
                out=w_n,
                in_=w_proj[:, n * 512:(n + 1) * 512]
                .rearrange("(k p) d -> p k d", p=P))
            for t in range(NT):
                mm_ps = psA.tile([P, 512], F32, tag="mm")
                for k in range(12):
                    nc.tensor.matmul(mm_ps, xT[:, k, t * P:(t + 1) * P],
                                     w_n[:, k, :], start=(k == 0), stop=(k == 11))
                sq_scr = work.tile([P, 512], F32, tag="sq_scr")
                nc.scalar.activation(sq_scr, mm_ps, AF.Square,
                                     accum_out=ssq[:, t, n:n + 1])
                nc.scalar.activation(q_raw[:, t, n * 512:(n + 1) * 512],
                                     mm_ps, AF.Copy)

        # RMS + rope per token tile (in-place on q_raw). The host permutes
        # wq/wk columns so each head is [even dims | odd dims]: the rope
        # halves are stride-1 (DVE 2x mode). rsq commutes with the rotation
        # so it's applied afterwards as one per-token scalar multiply.
        for t in range(NT):
            s01 = work.tile([P, 1], F32, tag="s01")
            nc.vector.tensor_tensor(s01, ssq[:, t, 0:1], ssq[:, t, 1:2], ALU.add)
            stot = work.tile([P, 1], F32, tag="stot")
            nc.vector.tensor_tensor(stot, s01, ssq[:, t, 2:3], ALU.add)
            sq_t = work.tile([P, 1], F32, tag="sq_t")
            nc.scalar.activation(sq_t, stot, AF.Sqrt, bias=eps_t,
                                 scale=1.0 / DIM)
            rsq = work.tile([P, 1], F32, tag="rsq")
            nc.vector.reciprocal(rsq, sq_t)
            cs_t = cos_sb[:, t, :]
            sn_t = sin_sb[:, t, :]
            cb = bass.AP(tensor=cs_t.tensor, offset=cs_t.offset,
                         ap=[cs_t.ap[0], [0, NH], cs_t.ap[1]])
            sbb = bass.AP(tensor=sn_t.tensor, offset=sn_t.offset,
                          ap=[sn_t.ap[0], [0, NH], sn_t.ap[1]])
            qh = q_raw[:, t, :].rearrange("p (h half c) -> p h half c",
                                          h=NH, half=2)
            qe = qh[:, :, 0, :]
            qo = qh[:, :, 1, :]
            tA = work.tile([P, NH, 64], BF, tag="tA")
            tB = work.tile([P, NH, 64], BF, tag="tB")
            tC = work.tile([P, NH, 64], BF, tag="tC")
            tD = work.tile([P, NH, 64], BF, tag="tD")
            nc.vector.tensor_tensor(tA, qe, cb, ALU.mult)
            nc.vector.tensor_tensor(tB, qo, sbb, ALU.mult)
            nc.vector.tensor_tensor(tC, qe, sbb, ALU.mult)
            nc.vector.tensor_tensor(tD, qo, cb, ALU.mult)
            nc.vector.tensor_tensor(qe, tA, tB, ALU.subtract)
            nc.vector.tensor_tensor(qo, tC, tD, ALU.add)
            nc.vector.tensor_scalar_mul(q_raw[:, t, :], q_raw[:, t, :], rsq)

        # transpose roped q/k into a per-head bf16 stage (transposes packed
        # 4-wide in PSUM), then DMA per route into the per-slot send buffers
        for h in range(12):
            stage_h = evict.tile([P, TQ], BF, tag="stage_h", bufs=3)
            tp5 = psT.tile([P, TQ], BF, tag="tpq")
            for t in range(NT):
                nc.tensor.transpose(tp5[:, t * P:(t + 1) * P],
                                    q_raw[:, t, h * P:(h + 1) * P], ident)
            nc.vector.tensor_copy(stage_h, tp5)
            for (d, sl) in _head_dests(h):
                nc.sync.dma_start(out=send_qk[sl][d, :, :], in_=stage_h)

        # ---------------- A2A: qk slot 0, then slot 1 ----------------
        nc.gpsimd.collective_compute(
            "AllToAll", ALU.bypass, replica_groups=[list(range(N_CORES))],
            ins=[send_qk0.opt()], outs=[recv_qk0.opt()])
        nc.gpsimd.collective_compute(
            "AllToAll", ALU.bypass, replica_groups=[list(range(N_CORES))],
            ins=[send_qk1.opt()], outs=[recv_qk1.opt()])

        # C: v projection [TQ, VHALF] into v_sb (evicted via ACT copy-cast),
        # then route columns per dest on the vector queue
        v_sb = res.tile([P, NT, VHALF], BF)
        for n0, nn in ((0, 512), (512, 256)):
            wv_n = wpool.tile([P, 12, 512], BF, tag="w_n")
            nc.sync.dma_start(
                out=wv_n[:, :, 0:nn],
                in_=wv_half[:, n0:n0 + nn]
                .rearrange("(k p) d -> p k d", p=P))
            for t in range(NT):
                mm_ps = psA.tile([P, 512], F32, tag="mm")
                for k in range(12):
                    nc.tensor.matmul(mm_ps[:, 0:nn], xT[:, k, t * P:(t + 1) * P],
                                     wv_n[:, k, 0:nn], start=(k == 0), stop=(k == 11))
                nc.scalar.activation(v_sb[:, t, n0:n0 + nn], mm_ps[:, 0:nn],
                                     AF.Copy)

        # v routing: local head column lh is head lh on a q-core and head
        # 6+lh on a k-core. Write BOTH halves' destination patterns
        # statically - a consumer only reads the shards of the 4 ranks of the
        # correct half for each slot, so wrong-half writes are dead data.
        for lh in range(6):
            dests = set(_head_dests(lh)) | set(_head_dests(6 + lh))
            for (d, sl) in sorted(dests):
                nc.scalar.dma_start(
                    out=send_v[d].rearrange("(a p) v -> p a v", p=P)
                    [:, :, sl * P:(sl + 1) * P],
                    in_=v_sb[:, :, lh * P:(lh + 1) * P])

    # ---------------- A2A: v (overlaps with the score pass) ----------
    nc.gpsimd.collective_compute(
        "AllToAll", ALU.bypass, replica_groups=[list(range(N_CORES))],
        ins=[send_v.opt()], outs=[recv_v.opt()])

    r_v2d = recv_v.rearrange("r t v -> (r t) v")

    # ---------------- Phase D: attention, 2 head slots ----------------
    with tc.tile_pool(name="attn", bufs=1) as attn, \
         tc.tile_pool(name="aev", bufs=2) as aev:

        qT = attn.tile([P, 2, NTL * P], BF)
        kT = attn.tile([P, 2, NTL * P], BF)
        Vc = attn.tile([P, 2, NTL, P], BF)
        PT = attn.tile([P, 2, PT_COLS], BF)      # all exp'd P^T tiles (10MB)
        PTS = attn.tile([P, 2, 4, 640], BF)      # per-group sum over kt of P^T
        rl_bc = attn.tile([P, 2, NTL * P], BF)   # 1/l broadcast to 128 rows

        for slot in range(2):
            for r in range(4):
                tb = 4 * r * P
                nl = QTILES[r] * P
                nc.sync.dma_start(out=qT[:, slot, tb:tb + nl],
                                  in_=recv_qk[slot][r, :, 0:nl])
                nc.sync.dma_start(out=kT[:, slot, tb:tb + nl],
                                  in_=recv_qk[slot][r + 4, :, 0:nl])

        # V loads (gpsimd queue: these sit after A2A-v, before A2A-o0)
        with nc.gpsimd.register("vr") as rr:
            for slot in range(2):
                for r in range(4):
                    idx = slot * 4 + r
                    nc.gpsimd.reg_load(rr, tab_sb[0:1, idx:idx + 1])
                    vrow = nc.gpsimd.snap(rr)
                    nc.gpsimd.dma_start(
                        out=Vc[:, slot, 4 * r:4 * r + QTILES[r], :],
                        in_=r_v2d[ds(vrow, QTILES[r] * P),
                                  slot * P:(slot + 1) * P]
                        .rearrange("(a p) d -> p a d", p=P))

        # Pass 1: scores + fused-span exp for both slots (needs only q/k);
        # DVE accumulates each group's P^T tiles into PTS, and as soon as a
        # group's sum is complete its softmax denominator is inverted (ACT)
        # and broadcast across partitions (ones-matmul) into rl_bc — so
        # pass 2 is a pure PV + multiply + stage pipeline.
        with tc.tile_pool(name="psS", bufs=2, space="PSUM") as psS, \
             tc.tile_pool(name="psR", bufs=2, space="PSUM") as psR:
            for slot in range(2):
                for span in EXP_SPANS:
                    span_len = sum(e[3] for e in span)
                    sp = psS.tile([P, 1536], F32, tag="sp")
                    for (t0, kt, so, n) in span:
                        col, c0, _ = PT_OFFS[(t0, kt)]
                        for (ja, jn) in _bank_chunks(so, n):
                            qa = c0 * P + (ja - so)
                            nc.tensor.matmul(sp[:, ja:ja + jn],
                                             kT[:, slot, kt * P:(kt + 1) * P],
                                             qT[:, slot, qa:qa + jn],
                                             start=True, stop=True)
                    col0 = PT_OFFS[(span[0][0], span[0][1])][0]
                    nc.scalar.activation(PT[:, slot, col0:col0 + span_len],
                                         sp[:, 0:span_len], AF.Exp, scale=SCALE)
                    for (t0, kt, so, n) in span:
                        col, c0, _ = PT_OFFS[(t0, kt)]
                        t1 = {0: 3, 4: 7, 8: 11, 12: 16}[t0]
                        gi = GROUPS.index((t0, t1))
                        pt = PT[:, slot, col:col + n]
                        if kt == 16:
                            # zero pad-key rows 32..128 (a base partition of
                            # 32 may span at most 32 rows)
                            nc.vector.tensor_scalar_mul(pt[32:64, :],
                                                        pt[32:64, :], 0.0)
                            nc.vector.tensor_scalar_mul(pt[64:P, :],
                                                        pt[64:P, :], 0.0)
                        # accumulate into the group sum (kt==0 initializes)
                        off = (c0 - t0) * P
                        dst = PTS[:, slot, gi, off:off + n]
                        if kt == 0:
                            nc.vector.tensor_copy(dst, pt)
                        else:
                            nc.vector.tensor_tensor(dst, dst, pt, ALU.add)
                        if kt == t1:
                            # group sum complete: 1/l, broadcast to 128 rows
                            ng = (t1 - t0 + 1) * P
                            for (ja, jn) in _chunks(ng):
                                rps = psR.tile([P, 512], F32, tag="rps")
                                nc.tensor.matmul(rps[0:1, 0:jn], ones_bf,
                                                 PTS[:, slot, gi, ja:ja + jn],
                                                 start=True, stop=True)
                                rl_f = aev.tile([1, 512], F32, tag="rl_f")
                                nc.vector.reciprocal(rl_f[:, 0:jn],
                                                     rps[0:1, 0:jn])
                                rl_bf = aev.tile([1, 512], BF, tag="rl_bf")
                                nc.vector.tensor_copy(rl_bf[:, 0:jn],
                                                      rl_f[:, 0:jn])
                                nc.tensor.matmul(rps[:, 0:jn], ones_row,
                                                 rl_bf[:, 0:jn],
                                                 start=True, stop=True)
                                nc.vector.tensor_copy(
                                    rl_bc[:, slot, t0 * P + ja:t0 * P + ja + jn],
                                    rps[:, 0:jn])

        # Pass 2: PV per group, multiply by 1/l, stage into destination token
        # blocks; each slot's output is exchanged while the next chunk of
        # compute runs.
        with tc.tile_pool(name="psO", bufs=2, space="PSUM") as psO:
            for slot in range(2):
                send_o = send_o0 if slot == 0 else send_o1
                for gi, (t0, t1) in enumerate(GROUPS):
                    ng = (t1 - t0 + 1) * P
                    g0 = t0 * P
                    oT_ps = psO.tile([P, 640], F32, tag="oT")
                    for kt in range(t1 + 1):
                        col, c0, n = PT_OFFS[(t0, kt)]
                        off = (c0 - t0) * P
                        pt = PT[:, slot, col:col + n]
                        # accumulation groups are per PSUM bank: a bank's last
                        # write happens at kt == its highest column tile
                        for (ja, jn) in _bank_chunks(off, n):
                            bank = ja // 512
                            fin = (kt == min(t1, t0 + 4 * bank + 3))
                            nc.tensor.matmul(oT_ps[:, ja:ja + jn],
                                             Vc[:, slot, kt, :],
                                             pt[:, ja - off:ja - off + jn],
                                             start=(kt == 0), stop=fin)
                    oT_n = aev.tile([P, 640], BF, tag="oT_n")
                    nc.vector.tensor_tensor(oT_n[:, 0:ng], oT_ps[:, 0:ng],
                                            rl_bc[:, slot, g0:g0 + ng],
                                            ALU.mult)
                    # slice into destination token blocks
                    for j in range(4):
                        a = max(g0, j * TOK_BLK)
                        b = min(g0 + ng, (j + 1) * TOK_BLK)
                        if a >= b:
                            continue
                        for dd in (j, j + 4):
                            nc.sync.dma_start(
                                out=send_o[dd, :, a - j * TOK_BLK:b - j * TOK_BLK],
                                in_=oT_n[:, a - g0:b - g0])
                # exchange this slot's output
                nc.gpsimd.collective_compute(
                    "AllToAll", ALU.bypass,
                    replica_groups=[list(range(N_CORES))],
                    ins=[(send_o0 if slot == 0 else send_o1).opt()],
                    outs=[(recv_o0 if slot == 0 else recv_o1).opt()])

    def head_src(h):
        return (h, 0) if h < 8 else (h - 8, 1)

    # ---------------- Phase E: output projection ----------------
    # Round 1 (heads 0-7, slot-0 data) accumulates to fp32 partials in SBUF
    # while the slot-1 exchange is still in flight; round 2 adds heads 8-11.
    with tc.tile_pool(name="oproj", bufs=1) as op, \
         tc.tile_pool(name="owork", bufs=3) as ow, \
         tc.tile_pool(name="psP", bufs=2, space="PSUM") as psP:

        oT_asm = op.tile([P, NH, TOK_BLK], BF)
        for h in range(NH):
            rk, sl = head_src(h)
            nc.sync.dma_start(out=oT_asm[:, h, :],
                              in_=(recv_o0 if sl == 0 else recv_o1)[rk, :, :])

        part = op.tile([P, 6, TOK_BLK], F32)
        for m in range(6):
            ps = psP.tile([P, TOK_BLK], F32, tag="psP")
            for (ja, jn) in _chunks(TOK_BLK):
                for k in range(8):
                    nc.tensor.matmul(ps[:, ja:ja + jn],
                                     wo_sb[:, k, m * P:(m + 1) * P],
                                     oT_asm[:, k, ja:ja + jn],
                                     start=(k == 0), stop=(k == 7))
            nc.vector.tensor_copy(part[:, m, :], ps)
        for m in range(6):
            ps = psP.tile([P, TOK_BLK], F32, tag="psP")
            for (ja, jn) in _chunks(TOK_BLK):
                for k in range(8, NH):
                    nc.tensor.matmul(ps[:, ja:ja + jn],
                                     wo_sb[:, k, m * P:(m + 1) * P],
                                     oT_asm[:, k, ja:ja + jn],
                                     start=(k == 8), stop=(k == NH - 1))
            oev = ow.tile([P, TOK_BLK], F32, tag="oev")
            nc.vector.tensor_tensor(oev, part[:, m, :], ps, ALU.add)
            nc.sync.dma_start(out=outT[m * P:(m + 1) * P, :], in_=oev)


# ======================= host side =======================

def _expected_mask():
    blk = np.arange(SPAD) // P
    return (blk[:, None] >= blk[None, :]) & (np.arange(SPAD)[None, :] < S)


def _host_prep(x, freqs, wq, wk, wv, wo):
    """Build the 8 per-core input maps (bf16 activations/weights)."""
    from ml_dtypes import bfloat16

    x_pad = np.zeros((SPAD, DIM), np.float32)
    x_pad[:S] = x[0]

    # rope angle table (pure gather from freqs) -> cos/sin
    t = np.arange(S)
    fi = t // (H_ * W_)
    hi = (t % (H_ * W_)) // W_
    wi = t % W_
    ang = np.zeros((SPAD, 64), np.float64)
    ang[:S, 0:22] = freqs[fi, 0:22]
    ang[:S, 22:43] = freqs[hi, 22:43]
    ang[:S, 43:64] = freqs[wi, 43:64]

    in_maps = []
    for c in range(N_CORES):
        qr = c % 4
        x_my = np.zeros((TQ, DIM), np.float32)
        x_my[:QLEN[qr]] = x_pad[QSTART[qr]:QSTART[qr] + QLEN[qr]]
        ang_q = ang[QSTART[qr]:QSTART[qr] + QLEN[qr]]
        cs_my = np.zeros((TQ, 128), np.float32)
        cs_my[:QLEN[qr], 0:64] = np.cos(ang_q)
        cs_my[:QLEN[qr], 64:128] = np.sin(ang_q)

        tabs = np.zeros((1, 32), np.int32)
        # consumer: v source rank row offsets per slot, per quarter
        for slot in range(2):
            head = _slot_head(c, slot)
            vbase = 0 if head < 6 else 4
            for r in range(4):
                tabs[0, slot * 4 + r] = (vbase + r) * TQ

        # permute q/k columns within each head to [even dims | odd dims] so
        # the on-device rope halves are contiguous (dot products and the
        # later transpose are invariant as long as q and k share the layout)
        perm = (np.arange(NH)[:, None] * HD
                + np.concatenate([np.arange(0, HD, 2),
                                  np.arange(1, HD, 2)])[None, :]).ravel()
        in_maps.append({
            "x_my": x_my.astype(bfloat16),
            "w_proj": np.ascontiguousarray(
                (wq if c < 4 else wk)[:, perm]).astype(bfloat16),
            "wv_half": np.ascontiguousarray(
                wv[:, :VHALF] if c < 4 else wv[:, VHALF:]).astype(bfloat16),
            "wo_slice": np.ascontiguousarray(
                wo[:, (c // 4) * NHALF:(c // 4 + 1) * NHALF]).astype(bfloat16),
            "cs_in": cs_my.astype(bfloat16),
            "tabs": tabs,
        })
    return in_maps


def get_nc():
    global _CACHED_NC
    if _CACHED_NC is None:
        _CACHED_NC = build_nc()
    return _CACHED_NC


def kernel(x, freqs, wq, bq, wk, bk, wv, bv, wo, bo, gq, gk,
           seq_lens, grid_sizes, mask, _run=None):
    x = np.asarray(x, np.float32)
    freqs = np.asarray(freqs, np.float32)
    wq, wk, wv, wo = (np.asarray(w, np.float32) for w in (wq, wk, wv, wo))

    assert x.shape == (1, S, DIM)
    assert int(np.asarray(seq_lens)[0]) == S, "teacher-forcing path not implemented"
    assert tuple(np.asarray(grid_sizes)[0]) == (F_, H_, W_)
    for b in (bq, bk, bv, bo):
        assert not np.any(np.asarray(b)), "nonzero bias not supported"
    for g in (gq, gk):
        assert np.all(np.asarray(g) == 1.0), "non-unit norm gain not supported"
    assert np.array_equal(np.asarray(mask), _expected_mask()), "unexpected mask"

    in_maps = _host_prep(x, freqs, wq, wk, wv, wo)

    if _run is None:
        from concourse.bass_utils import run_bass_kernel_spmd
        nc = get_nc()
        res = run_bass_kernel_spmd(nc, in_maps, list(range(N_CORES)))
        outs = [res.results[c]["outT"] for c in range(N_CORES)]
    else:
        outs = _run(in_maps)

    out = np.zeros((SPAD, DIM), np.float32)
    for c in range(N_CORES):
        r0 = (c % 4) * TOK_BLK
        n0 = (c // 4) * NHALF
        out[r0:r0 + TOK_BLK, n0:n0 + NHALF] = outs[c].T
    return out[:S][None]
